# revision 1
# baseline (speedup 1.0000x reference)
"""Trainium2 Bass kernel for a 2-layer GCN + sigmoid similarity matrix.

Model (see reference):
    h1 = relu(gcn_conv(x, W1, b1));  h2 = relu(gcn_conv(h1, W2, b2))
    out = sigmoid(h2 @ h2.T)                               # [8192, 8192]

gcn_conv(x, W, b) with self-loops and symmetric deg^{-1/2} norm factorizes:
    h  = x @ W
    out[d] = dinv[d] * sum_s Ahat[s, d] * (dinv[s] * h[s]) + b
where Ahat = edge-count matrix + I and dinv = rsqrt(indeg + 1).

Distribution over 8 NeuronCores (dst-sharded, per the sharding hint):
  - Every core computes hs = dinv * (x @ W1) for ALL nodes (cheap, replicated).
  - Ahat is densified per core as the [8192 src, 1024 dst] column shard, stored
    fp8_e4m3 (exact small integer counts) -> 8.4MB resident in SBUF.
  - Aggregation is a PE matmul: aggT[f, d] = sum_s hs[s, f] * Ahat[s, d],
    accumulated over 64 src chunks of 128 (lhsT = hs chunk bf16, rhs = A fp8).
  - Layer outputs stay feature-major ("T-form", [64 feats, nodes]); an
    AllGather of the [64, 1024] shard yields the full [64, 8192] table, whose
    [64, 128] column slices are directly the next layer's stationary operands.
  - Final phase: each core computes its [1024, 8192] block of
    sigmoid(h2 @ h2.T) (PE matmul K=64 + ScalarE sigmoid) and DMAs it out.
"""

import os
import sys

# bass/concourse toolchain location (not a problem-statement file)
for _p in ("/opt/trn_rl_repo", "/root/.axon_site/_ro/trn_rl_repo"):
    if os.path.isdir(_p) and _p not in sys.path:
        sys.path.insert(0, _p)
        break

# A cpu-forced JAX would hide the axon-tunneled NeuronCores this kernel needs.
if os.environ.get("JAX_PLATFORMS", "").strip().lower() in ("cpu",):
    os.environ.pop("JAX_PLATFORMS")

import numpy as np
import ml_dtypes

import concourse.bass as bass
import concourse.bacc as bacc
import concourse.mybir as mybir
from concourse import tile
from concourse.bass_utils import run_bass_kernel_spmd

N = 8192          # nodes
E = 262144        # edges
IN_DIM = 128
HID = 64
CORES = 8
DSH = N // CORES  # dst shard size (1024)
NCH = N // 128    # src chunks of 128 (64)

F32 = mybir.dt.float32
BF16 = mybir.dt.bfloat16
FP8 = mybir.dt.float8e4
AF = mybir.ActivationFunctionType
ALU = mybir.AluOpType

_COMPILED = {}


def _build_program():
    nc = bacc.Bacc("TRN2", target_bir_lowering=False, debug=False,
                   num_devices=CORES)

    # ---- I/O ----
    xT_d = nc.dram_tensor("xT", [IN_DIM, N], F32, kind="ExternalInput")
    A_d = nc.dram_tensor("A", [128, NCH * DSH], FP8, kind="ExternalInput")
    degt_d = nc.dram_tensor("degt", [128, NCH], F32, kind="ExternalInput")
    degb_d = nc.dram_tensor("degb", [HID, DSH], F32, kind="ExternalInput")
    W1_d = nc.dram_tensor("W1", [IN_DIM, HID], F32, kind="ExternalInput")
    W2_d = nc.dram_tensor("W2", [HID, HID], F32, kind="ExternalInput")
    b1_d = nc.dram_tensor("b1", [HID, 1], F32, kind="ExternalInput")
    b2_d = nc.dram_tensor("b2", [HID, 1], F32, kind="ExternalInput")
    out_d = nc.dram_tensor("out", [DSH, N], F32, kind="ExternalOutput")

    def rsqrt_newton(pool, deg_t, p, fd):
        """dinv = rsqrt(deg): DVE reciprocal + ACT sqrt + one Newton step."""
        r = pool.tile([p, fd], F32, name=f"rs_r_{p}_{fd}")
        y = pool.tile([p, fd], F32, name=f"rs_y_{p}_{fd}")
        t = pool.tile([p, fd], F32, name=f"rs_t_{p}_{fd}")
        nc.vector.reciprocal(r[:], deg_t[:])
        nc.scalar.activation(y[:], r[:], AF.Sqrt)
        nc.vector.tensor_mul(t[:], y[:], y[:])
        nc.vector.tensor_mul(t[:], t[:], deg_t[:])
        nc.vector.tensor_scalar(t[:], t[:], -0.5, 1.5, ALU.mult, ALU.add)
        nc.vector.tensor_mul(y[:], y[:], t[:])
        return y

    with tile.TileContext(nc) as tc:
        with tc.tile_pool(name="const", bufs=1) as cpool, \
             tc.tile_pool(name="amat", bufs=1) as apool, \
             tc.tile_pool(name="hst", bufs=1) as hpool, \
             tc.tile_pool(name="dram", bufs=1, space="DRAM") as dpool:

            # ---- resident tiles / constants ----
            A_sb = apool.tile([128, NCH * DSH], FP8)
            nc.sync.dma_start(A_sb[:], A_d[:])

            W1_t = cpool.tile([IN_DIM, HID], F32)
            W2_t = cpool.tile([HID, HID], F32)
            b1_t = cpool.tile([HID, 1], F32)
            b2_t = cpool.tile([HID, 1], F32)
            degt_t = cpool.tile([128, NCH], F32)
            degb_t = cpool.tile([HID, DSH], F32)
            nc.sync.dma_start(W1_t[:], W1_d[:])
            nc.sync.dma_start(W2_t[:], W2_d[:])
            nc.sync.dma_start(b1_t[:], b1_d[:])
            nc.sync.dma_start(b2_t[:], b2_d[:])
            nc.sync.dma_start(degt_t[:], degt_d[:])
            nc.sync.dma_start(degb_t[:], degb_d[:])

            dinvt = rsqrt_newton(cpool, degt_t, 128, NCH)   # [128, 64]
            dinvb = rsqrt_newton(cpool, degb_t, HID, DSH)   # [64, 1024]

            h1T_shard = cpool.tile([HID, DSH], F32)
            h2T_shard = cpool.tile([HID, DSH], F32)

            def gcn_layer(hs_sb, W_t, b_t, hT_out, make_lhsT):
                """hs_sb [128, NCH*HID] bf16 <- dinv*(prev @ W); then
                hT_out [64, DSH] <- relu(dinv_d * (hs.T @ A) + b)."""
                with tc.tile_pool(name="ph_psum", bufs=4, space="PSUM") as pp:
                    for c in range(NCH):
                        ph = pp.tile([128, HID], F32, tag="ph")
                        nc.tensor.matmul(ph[:], make_lhsT(c), W_t[:],
                                         start=True, stop=True)
                        nc.vector.tensor_scalar(
                            hs_sb[:, c * HID:(c + 1) * HID], ph[:],
                            dinvt[:, c:c + 1], None, ALU.mult)
                with tc.tile_pool(name="ag_psum", bufs=2, space="PSUM") as gp, \
                     tc.tile_pool(name="ag_tmp", bufs=2) as tp:
                    for h in range(2):
                        pg = gp.tile([HID, 512], F32, tag="pg")
                        for c in range(NCH):
                            nc.tensor.matmul(
                                pg[:],
                                hs_sb[:, c * HID:(c + 1) * HID],
                                A_sb[:, c * DSH + h * 512: c * DSH + (h + 1) * 512],
                                start=(c == 0), stop=(c == NCH - 1))
                        tmp = tp.tile([HID, 512], F32, tag="tmp")
                        nc.vector.tensor_mul(tmp[:], pg[:],
                                             dinvb[:, h * 512:(h + 1) * 512])
                        nc.scalar.activation(hT_out[:, h * 512:(h + 1) * 512],
                                             tmp[:], AF.Relu, bias=b_t[:])

            def allgather_T(hT_sh, full_sb, idx):
                agin = dpool.tile([HID, DSH], F32, name=f"agin{idx}")
                agout = dpool.tile([CORES * HID, DSH], F32,
                                   addr_space="Shared", name=f"agout{idx}")
                nc.gpsimd.dma_start(agin[:], hT_sh[:])
                nc.gpsimd.collective_compute(
                    "AllGather", ALU.bypass,
                    replica_groups=[list(range(CORES))],
                    ins=[agin.opt()], outs=[agout.opt()])
                nc.sync.dma_start(
                    full_sb.rearrange("f (r j) -> f r j", j=DSH),
                    agout.rearrange("(r f) j -> f r j", f=HID))

            # ---- layer 1 ----
            with tc.tile_pool(name="l1", bufs=1) as l1pool:
                xT_sb = l1pool.tile([IN_DIM, N], F32)
                nc.sync.dma_start(xT_sb[:], xT_d[:])
                hs1 = l1pool.tile([128, NCH * HID], BF16)
                gcn_layer(hs1, W1_t, b1_t, h1T_shard,
                          lambda c: xT_sb[:, c * 128:(c + 1) * 128])

            # ---- allgather h1, layer 2 ----
            with tc.tile_pool(name="l2", bufs=1) as l2pool:
                h1T_full = l2pool.tile([HID, N], F32)
                allgather_T(h1T_shard, h1T_full, 1)
                hs2 = l2pool.tile([128, NCH * HID], BF16)
                gcn_layer(hs2, W2_t, b2_t, h2T_shard,
                          lambda c: h1T_full[:, c * 128:(c + 1) * 128])

            # ---- allgather h2, similarity + sigmoid ----
            with tc.tile_pool(name="sim", bufs=1) as spool, \
                 tc.tile_pool(name="sim_psum", bufs=4, space="PSUM") as sp, \
                 tc.tile_pool(name="stage", bufs=3) as stpool:
                h2T_full = spool.tile([HID, N], F32)
                allgather_T(h2T_shard, h2T_full, 2)
                for m in range(DSH // 128):
                    lhsT = h2T_shard[:, m * 128:(m + 1) * 128]
                    for q in range(4):
                        st = stpool.tile([128, 2048], F32, tag="st")
                        for k in range(4):
                            j = q * 4 + k
                            ps = sp.tile([128, 512], F32, tag="ps")
                            nc.tensor.matmul(
                                ps[:], lhsT,
                                h2T_full[:, j * 512:(j + 1) * 512],
                                start=True, stop=True)
                            nc.scalar.activation(st[:, k * 512:(k + 1) * 512],
                                                 ps[:], AF.Sigmoid)
                        nc.sync.dma_start(
                            out_d[m * 128:(m + 1) * 128,
                                  q * 2048:(q + 1) * 2048], st[:])

    nc.compile()
    return nc


def _get_program():
    if "nc" not in _COMPILED:
        _COMPILED["nc"] = _build_program()
    return _COMPILED["nc"]


def _prep_inputs(x, edge_index, W1, b1, W2, b2):
    x = np.asarray(x, np.float32)
    ei = np.asarray(edge_index)
    src = ei[0].astype(np.int64)
    dst = ei[1].astype(np.int64)

    deg = (np.bincount(dst, minlength=N) + 1).astype(np.float32)
    xT = np.ascontiguousarray(x.T)                                 # [128, N]
    degt = np.ascontiguousarray(deg.reshape(NCH, 128).T)           # [128, 64]
    W1c = np.ascontiguousarray(np.asarray(W1, np.float32))
    W2c = np.ascontiguousarray(np.asarray(W2, np.float32))
    b1c = np.asarray(b1, np.float32).reshape(HID, 1).copy()
    b2c = np.asarray(b2, np.float32).reshape(HID, 1).copy()

    in_maps = []
    for i in range(CORES):
        lo = i * DSH
        sel = (dst >= lo) & (dst < lo + DSH)
        flat = src[sel] * DSH + (dst[sel] - lo)
        cnt = np.bincount(flat, minlength=N * DSH).reshape(N, DSH)
        cnt[np.arange(lo, lo + DSH), np.arange(DSH)] += 1          # + I shard
        # SBUF layout: partition p holds src rows {c*128+p}, free = c*DSH + d
        A8 = np.ascontiguousarray(
            cnt.reshape(NCH, 128, DSH).transpose(1, 0, 2)
        ).astype(ml_dtypes.float8_e4m3).reshape(128, NCH * DSH)
        degb = np.ascontiguousarray(
            np.broadcast_to(deg[lo:lo + DSH][None, :], (HID, DSH)))
        in_maps.append({
            "xT": xT, "A": A8, "degt": degt, "degb": degb,
            "W1": W1c, "W2": W2c, "b1": b1c, "b2": b2c,
        })
    return in_maps


def kernel(x, edge_index, W1, b1, W2, b2, _trace=False, _trace_kwargs=None):
    nc = _get_program()
    in_maps = _prep_inputs(x, edge_index, W1, b1, W2, b2)
    res = run_bass_kernel_spmd(nc, in_maps, core_ids=list(range(CORES)),
                               trace=_trace, **(_trace_kwargs or {}))
    out = np.concatenate([res.results[i]["out"] for i in range(CORES)], axis=0)
    if _trace:
        kernel._last_results = res
    return np.ascontiguousarray(out, dtype=np.float32)


# revision 2
# speedup vs baseline: 1.5118x; 1.5118x over previous
"""Trainium2 Bass kernel for a 2-layer GCN + sigmoid similarity matrix.

Model (see reference):
    h1 = relu(gcn_conv(x, W1, b1));  h2 = relu(gcn_conv(h1, W2, b2))
    out = sigmoid(h2 @ h2.T)                               # [8192, 8192]

gcn_conv(x, W, b) with self-loops and symmetric deg^{-1/2} norm factorizes:
    h  = x @ W
    out[d] = dinv[d] * sum_s Ahat[s, d] * (dinv[s] * h[s]) + b
where Ahat = edge-count matrix + I and dinv = rsqrt(indeg + 1).

Distribution over 8 NeuronCores (dst-sharded, per the sharding hint):
  - Every core computes hs = dinv * (x @ W1) for ALL nodes (cheap, replicated).
  - Ahat is densified per core as the [8192 src, 1024 dst] column shard, stored
    fp8_e4m3 (exact small integer counts) -> 8.4MB resident in SBUF.
  - Aggregation is a PE matmul: aggT[f, d] = sum_s hs[s, f] * Ahat[s, d],
    accumulated over 64 src chunks of 128 (lhsT = hs chunk bf16, rhs = A fp8).
  - Layer outputs stay feature-major ("T-form", [64 feats, nodes], bf16); an
    AllGather of the [64, 1024] shard yields the full [64, 8192] table, whose
    [64, 128] column slices are directly the next layer's stationary operands.
  - Final phase: each core computes its [1024, 8192] block of
    sigmoid(h2 @ h2.T) (bf16 PE matmul K=64 + ScalarE sigmoid) and DMAs it.

All TensorEngine operands are bf16/fp8 (fp32 matmul runs as two PE passes and
dominated the first profile); PSUM accumulation and the sigmoid are fp32.
"""

import os
import sys

# bass/concourse toolchain location (not a problem-statement file)
for _p in ("/opt/trn_rl_repo", "/root/.axon_site/_ro/trn_rl_repo"):
    if os.path.isdir(_p) and _p not in sys.path:
        sys.path.insert(0, _p)
        break

# A cpu-forced JAX would hide the axon-tunneled NeuronCores this kernel needs.
if os.environ.get("JAX_PLATFORMS", "").strip().lower() in ("cpu",):
    os.environ.pop("JAX_PLATFORMS")

import numpy as np
import ml_dtypes

import concourse.bass as bass
import concourse.bacc as bacc
import concourse.mybir as mybir
from concourse import tile
from concourse.bass_utils import run_bass_kernel_spmd

N = 8192          # nodes
E = 262144        # edges
IN_DIM = 128
HID = 64
CORES = 8
DSH = N // CORES  # dst shard size (1024)
NCH = N // 128    # src chunks of 128 (64)

F32 = mybir.dt.float32
BF16 = mybir.dt.bfloat16
FP8 = mybir.dt.float8e4
AF = mybir.ActivationFunctionType
ALU = mybir.AluOpType

_COMPILED = {}


def _build_program():
    nc = bacc.Bacc("TRN2", target_bir_lowering=False, debug=False,
                   num_devices=CORES)

    # ---- I/O ----
    xT_d = nc.dram_tensor("xT", [IN_DIM, N], BF16, kind="ExternalInput")
    A_d = nc.dram_tensor("A", [128, NCH * DSH], FP8, kind="ExternalInput")
    degt_d = nc.dram_tensor("degt", [128, NCH], F32, kind="ExternalInput")
    degb_d = nc.dram_tensor("degb", [HID, DSH], F32, kind="ExternalInput")
    W1_d = nc.dram_tensor("W1", [IN_DIM, HID], BF16, kind="ExternalInput")
    W2_d = nc.dram_tensor("W2", [HID, HID], BF16, kind="ExternalInput")
    b1_d = nc.dram_tensor("b1", [HID, 1], F32, kind="ExternalInput")
    b2_d = nc.dram_tensor("b2", [HID, 1], F32, kind="ExternalInput")
    out_d = nc.dram_tensor("out", [DSH, N], F32, kind="ExternalOutput")

    def rsqrt_newton(pool, deg_t, p, fd):
        """dinv = rsqrt(deg): DVE reciprocal + ACT sqrt + one Newton step."""
        r = pool.tile([p, fd], F32, name=f"rs_r_{p}_{fd}")
        y = pool.tile([p, fd], F32, name=f"rs_y_{p}_{fd}")
        t = pool.tile([p, fd], F32, name=f"rs_t_{p}_{fd}")
        nc.vector.reciprocal(r[:], deg_t[:])
        nc.scalar.activation(y[:], r[:], AF.Sqrt)
        nc.vector.tensor_mul(t[:], y[:], y[:])
        nc.vector.tensor_mul(t[:], t[:], deg_t[:])
        nc.vector.tensor_scalar(t[:], t[:], -0.5, 1.5, ALU.mult, ALU.add)
        nc.vector.tensor_mul(y[:], y[:], t[:])
        return y

    with tile.TileContext(nc) as tc:
        with tc.tile_pool(name="const", bufs=1) as cpool, \
             tc.tile_pool(name="amat", bufs=1) as apool, \
             tc.tile_pool(name="dram", bufs=1, space="DRAM") as dpool:

            # ---- resident tiles / constants ----
            # A in 8 pieces so layer-1 aggregation can start on early chunks.
            A_sb = apool.tile([128, NCH * DSH], FP8)
            APIECE = NCH // 8
            for a in range(8):
                sl = slice(a * APIECE * DSH, (a + 1) * APIECE * DSH)
                nc.sync.dma_start(A_sb[:, sl], A_d[:, sl])

            W1_t = cpool.tile([IN_DIM, HID], BF16)
            W2_t = cpool.tile([HID, HID], BF16)
            b1_t = cpool.tile([HID, 1], F32)
            b2_t = cpool.tile([HID, 1], F32)
            degt_t = cpool.tile([128, NCH], F32)
            degb_t = cpool.tile([HID, DSH], F32)
            nc.sync.dma_start(W1_t[:], W1_d[:])
            nc.sync.dma_start(W2_t[:], W2_d[:])
            nc.sync.dma_start(b1_t[:], b1_d[:])
            nc.sync.dma_start(b2_t[:], b2_d[:])
            nc.sync.dma_start(degt_t[:], degt_d[:])
            nc.sync.dma_start(degb_t[:], degb_d[:])

            dinvt = rsqrt_newton(cpool, degt_t, 128, NCH)   # [128, 64]
            dinvb = rsqrt_newton(cpool, degb_t, HID, DSH)   # [64, 1024]

            h1T_shard = cpool.tile([HID, DSH], BF16)
            h2T_shard = cpool.tile([HID, DSH], BF16)

            def gcn_layer(hs_sb, W_t, b_t, hT_out, make_lhsT):
                """hs_sb [128, NCH*HID] bf16 <- dinv*(prev @ W); then
                hT_out [64, DSH] bf16 <- relu(dinv_d * (hs.T @ A) + b)."""
                with tc.tile_pool(name="ph_psum", bufs=4, space="PSUM") as pp:
                    for c in range(NCH):
                        ph = pp.tile([128, HID], F32, tag="ph")
                        nc.tensor.matmul(ph[:], make_lhsT(c), W_t[:],
                                         start=True, stop=True)
                        nc.vector.tensor_scalar(
                            hs_sb[:, c * HID:(c + 1) * HID], ph[:],
                            dinvt[:, c:c + 1], None, ALU.mult)
                with tc.tile_pool(name="ag_psum", bufs=2, space="PSUM") as gp, \
                     tc.tile_pool(name="ag_tmp", bufs=2) as tp:
                    for h in range(2):
                        pg = gp.tile([HID, 512], F32, tag="pg")
                        for c in range(NCH):
                            nc.tensor.matmul(
                                pg[:],
                                hs_sb[:, c * HID:(c + 1) * HID],
                                A_sb[:, c * DSH + h * 512: c * DSH + (h + 1) * 512],
                                start=(c == 0), stop=(c == NCH - 1))
                        tmp = tp.tile([HID, 512], F32, tag="tmp")
                        nc.vector.tensor_mul(tmp[:], pg[:],
                                             dinvb[:, h * 512:(h + 1) * 512])
                        nc.scalar.activation(hT_out[:, h * 512:(h + 1) * 512],
                                             tmp[:], AF.Relu, bias=b_t[:])

            def allgather_T(hT_sh, full_sb, idx):
                agin = dpool.tile([HID, DSH], BF16, name=f"agin{idx}")
                agout = dpool.tile([CORES * HID, DSH], BF16,
                                   addr_space="Shared", name=f"agout{idx}")
                nc.gpsimd.dma_start(agin[:], hT_sh[:])
                nc.gpsimd.collective_compute(
                    "AllGather", ALU.bypass,
                    replica_groups=[list(range(CORES))],
                    ins=[agin.opt()], outs=[agout.opt()])
                nc.sync.dma_start(
                    full_sb.rearrange("f (r j) -> f r j", j=DSH),
                    agout.rearrange("(r f) j -> f r j", f=HID))

            # ---- layer 1 ----
            with tc.tile_pool(name="l1", bufs=1) as l1pool:
                xT_sb = l1pool.tile([IN_DIM, N], BF16)
                for a in range(4):
                    sl = slice(a * (N // 4), (a + 1) * (N // 4))
                    nc.sync.dma_start(xT_sb[:, sl], xT_d[:, sl])
                hs1 = l1pool.tile([128, NCH * HID], BF16)
                gcn_layer(hs1, W1_t, b1_t, h1T_shard,
                          lambda c: xT_sb[:, c * 128:(c + 1) * 128])

            # ---- allgather h1, layer 2 ----
            with tc.tile_pool(name="l2", bufs=1) as l2pool:
                h1T_full = l2pool.tile([HID, N], BF16)
                allgather_T(h1T_shard, h1T_full, 1)
                hs2 = l2pool.tile([128, NCH * HID], BF16)
                gcn_layer(hs2, W2_t, b2_t, h2T_shard,
                          lambda c: h1T_full[:, c * 128:(c + 1) * 128])

            # ---- allgather h2, similarity + sigmoid ----
            with tc.tile_pool(name="sim", bufs=1) as spool, \
                 tc.tile_pool(name="sim_psum", bufs=3, space="PSUM") as sp, \
                 tc.tile_pool(name="stage", bufs=3) as stpool:
                h2T_full = spool.tile([HID, N], BF16)
                allgather_T(h2T_shard, h2T_full, 2)
                for m in range(DSH // 128):
                    lhsT = h2T_shard[:, m * 128:(m + 1) * 128]
                    for q in range(4):
                        st = stpool.tile([128, 2048], F32, tag="st")
                        for k in range(2):
                            ps = sp.tile([128, 1024], F32, tag="ps")
                            for u in range(2):
                                j = q * 2048 + k * 1024 + u * 512
                                nc.tensor.matmul(
                                    ps[:, u * 512:(u + 1) * 512], lhsT,
                                    h2T_full[:, j:j + 512],
                                    start=True, stop=True)
                            nc.scalar.activation(
                                st[:, k * 1024:(k + 1) * 1024], ps[:],
                                AF.Sigmoid)
                        nc.sync.dma_start(
                            out_d[m * 128:(m + 1) * 128,
                                  q * 2048:(q + 1) * 2048], st[:])

    nc.compile()
    return nc


def _get_program():
    if "nc" not in _COMPILED:
        _COMPILED["nc"] = _build_program()
    return _COMPILED["nc"]


def _prep_inputs(x, edge_index, W1, b1, W2, b2):
    x = np.asarray(x, np.float32)
    ei = np.asarray(edge_index)
    src = ei[0].astype(np.int64)
    dst = ei[1].astype(np.int64)

    deg = (np.bincount(dst, minlength=N) + 1).astype(np.float32)
    xT = np.ascontiguousarray(x.T).astype(ml_dtypes.bfloat16)      # [128, N]
    degt = np.ascontiguousarray(deg.reshape(NCH, 128).T)           # [128, 64]
    W1c = np.asarray(W1, np.float32).astype(ml_dtypes.bfloat16)
    W2c = np.asarray(W2, np.float32).astype(ml_dtypes.bfloat16)
    b1c = np.asarray(b1, np.float32).reshape(HID, 1).copy()
    b2c = np.asarray(b2, np.float32).reshape(HID, 1).copy()

    in_maps = []
    for i in range(CORES):
        lo = i * DSH
        sel = (dst >= lo) & (dst < lo + DSH)
        flat = src[sel] * DSH + (dst[sel] - lo)
        cnt = np.bincount(flat, minlength=N * DSH).reshape(N, DSH)
        cnt[np.arange(lo, lo + DSH), np.arange(DSH)] += 1          # + I shard
        # SBUF layout: partition p holds src rows {c*128+p}, free = c*DSH + d
        A8 = np.ascontiguousarray(
            cnt.reshape(NCH, 128, DSH).transpose(1, 0, 2)
        ).astype(ml_dtypes.float8_e4m3).reshape(128, NCH * DSH)
        degb = np.ascontiguousarray(
            np.broadcast_to(deg[lo:lo + DSH][None, :], (HID, DSH)))
        in_maps.append({
            "xT": xT, "A": A8, "degt": degt, "degb": degb,
            "W1": W1c, "W2": W2c, "b1": b1c, "b2": b2c,
        })
    return in_maps


def kernel(x, edge_index, W1, b1, W2, b2, _trace=False, _trace_kwargs=None):
    nc = _get_program()
    in_maps = _prep_inputs(x, edge_index, W1, b1, W2, b2)
    res = run_bass_kernel_spmd(nc, in_maps, core_ids=list(range(CORES)),
                               trace=_trace, **(_trace_kwargs or {}))
    out = np.concatenate([res.results[i]["out"] for i in range(CORES)], axis=0)
    if _trace:
        kernel._last_results = res
    return np.ascontiguousarray(out, dtype=np.float32)


# revision 6
# speedup vs baseline: 1.6853x; 1.1148x over previous
"""Trainium2 Bass kernel for a 2-layer GCN + sigmoid similarity matrix.

Model (see reference):
    h1 = relu(gcn_conv(x, W1, b1));  h2 = relu(gcn_conv(h1, W2, b2))
    out = sigmoid(h2 @ h2.T)                               # [8192, 8192]

gcn_conv(x, W, b) with self-loops and symmetric deg^{-1/2} norm factorizes:
    h  = x @ W
    out[d] = dinv[d] * sum_s Ahat[s, d] * (dinv[s] * h[s]) + b
where Ahat = edge-count matrix + I and dinv = rsqrt(indeg + 1).

Distribution over 8 NeuronCores (dst-sharded, per the sharding hint):
  - Every core computes hs = dinv * (x @ W1) for ALL nodes (cheap, replicated).
  - Ahat is densified per core as the [8192 src, 1024 dst] column shard, stored
    fp8_e4m3 (exact small integer counts) -> 8.4MB resident in SBUF.
  - Aggregation is a PE matmul: aggT[f, d] = sum_s hs[s, f] * Ahat[s, d],
    accumulated over 64 src chunks of 128 (lhsT = hs chunk bf16, rhs = A fp8).
  - Layer outputs stay feature-major ("T-form", [64 feats, nodes], bf16); an
    AllGather of the [64, 1024] shard yields the full [64, 8192] table, whose
    [64, 128] column slices are directly the next layer's stationary operands.
  - Final phase: each core computes its [1024, 8192] block of
    sigmoid(h2 @ h2.T) (bf16 PE matmul K=64 + ScalarE sigmoid) and DMAs it.

All TensorEngine operands are bf16/fp8 (fp32 matmul runs as two PE passes and
dominated the first profile); PSUM accumulation and the sigmoid are fp32.
"""

import os
import sys

# bass/concourse toolchain location (not a problem-statement file)
for _p in ("/opt/trn_rl_repo", "/root/.axon_site/_ro/trn_rl_repo"):
    if os.path.isdir(_p) and _p not in sys.path:
        sys.path.insert(0, _p)
        break

# A cpu-forced JAX would hide the axon-tunneled NeuronCores this kernel needs.
if os.environ.get("JAX_PLATFORMS", "").strip().lower() in ("cpu",):
    os.environ.pop("JAX_PLATFORMS")

import numpy as np
import ml_dtypes

import concourse.bass as bass
import concourse.bacc as bacc
import concourse.mybir as mybir
from concourse import tile
from concourse.bass_utils import run_bass_kernel_spmd

N = 8192          # nodes
E = 262144        # edges
IN_DIM = 128
HID = 64
CORES = 8
DSH = N // CORES  # dst shard size (1024)
NCH = N // 128    # src chunks of 128 (64)

F32 = mybir.dt.float32
BF16 = mybir.dt.bfloat16
FP8 = mybir.dt.float8e4
AF = mybir.ActivationFunctionType
ALU = mybir.AluOpType

_COMPILED = {}


def _build_program():
    nc = bacc.Bacc("TRN2", target_bir_lowering=False, debug=False,
                   num_devices=CORES)

    # ---- I/O ----
    xT_d = nc.dram_tensor("xT", [IN_DIM, N], BF16, kind="ExternalInput")
    A_d = nc.dram_tensor("A", [128, NCH * DSH], FP8, kind="ExternalInput")
    degt_d = nc.dram_tensor("degt", [128, NCH], F32, kind="ExternalInput")
    degb_d = nc.dram_tensor("degb", [HID, DSH], F32, kind="ExternalInput")
    W1_d = nc.dram_tensor("W1", [IN_DIM, HID], BF16, kind="ExternalInput")
    W2_d = nc.dram_tensor("W2", [HID, HID], BF16, kind="ExternalInput")
    b1_d = nc.dram_tensor("b1", [HID, 1], F32, kind="ExternalInput")
    b2_d = nc.dram_tensor("b2", [HID, 1], F32, kind="ExternalInput")
    out_d = nc.dram_tensor("out", [DSH, N], F32, kind="ExternalOutput")

    def rsqrt_newton(pool, deg_t, p, fd):
        """dinv = rsqrt(deg): DVE reciprocal + ACT sqrt + one Newton step."""
        r = pool.tile([p, fd], F32, name=f"rs_r_{p}_{fd}")
        y = pool.tile([p, fd], F32, name=f"rs_y_{p}_{fd}")
        t = pool.tile([p, fd], F32, name=f"rs_t_{p}_{fd}")
        nc.vector.reciprocal(r[:], deg_t[:])
        nc.scalar.activation(y[:], r[:], AF.Sqrt)
        nc.vector.tensor_mul(t[:], y[:], y[:])
        nc.vector.tensor_mul(t[:], t[:], deg_t[:])
        nc.vector.tensor_scalar(t[:], t[:], -0.5, 1.5, ALU.mult, ALU.add)
        nc.vector.tensor_mul(y[:], y[:], t[:])
        return y

    with tile.TileContext(nc) as tc:
        with tc.tile_pool(name="const", bufs=1) as cpool, \
             tc.tile_pool(name="amat", bufs=1) as apool, \
             tc.tile_pool(name="dram", bufs=1, space="DRAM") as dpool:

            # ---- resident tiles / constants ----
            # A in 8 pieces so layer-1 aggregation can start on early chunks.
            A_sb = apool.tile([128, NCH * DSH], FP8)
            APIECE = NCH // 8
            for a in range(8):
                sl = slice(a * APIECE * DSH, (a + 1) * APIECE * DSH)
                nc.sync.dma_start(A_sb[:, sl], A_d[:, sl])

            W1_t = cpool.tile([IN_DIM, HID], BF16)
            W2_t = cpool.tile([HID, HID], BF16)
            b1_t = cpool.tile([HID, 1], F32)
            b2_t = cpool.tile([HID, 1], F32)
            degt_t = cpool.tile([128, NCH], F32)
            degb_t = cpool.tile([HID, DSH], F32)
            nc.sync.dma_start(W1_t[:], W1_d[:])
            nc.sync.dma_start(W2_t[:], W2_d[:])
            nc.sync.dma_start(b1_t[:], b1_d[:])
            nc.sync.dma_start(b2_t[:], b2_d[:])
            nc.sync.dma_start(degt_t[:], degt_d[:])
            nc.sync.dma_start(degb_t[:], degb_d[:])

            # Tiny warm-up AllGather: absorbs the first-collective ncfw wakeup
            # and trigger latency while the input DMAs stream in.
            warm_sb = cpool.tile([64, 16], BF16)
            nc.gpsimd.memset(warm_sb[:], 0.0)
            warm_in = dpool.tile([64, 16], BF16)
            warm_out = dpool.tile([CORES * 64, 16], BF16, addr_space="Shared")
            nc.gpsimd.dma_start(warm_in[:], warm_sb[:])
            nc.gpsimd.collective_compute(
                "AllGather", ALU.bypass,
                replica_groups=[list(range(CORES))],
                ins=[warm_in.opt()], outs=[warm_out.opt()])

            dinvt = rsqrt_newton(cpool, degt_t, 128, NCH)   # [128, 64]
            dinvb = rsqrt_newton(cpool, degb_t, HID, DSH)   # [64, 1024]

            h1T_shard = cpool.tile([HID, DSH], BF16)
            h2T_shard = cpool.tile([HID, DSH], BF16)

            def gcn_layer(hs_sb, W_t, b_t, hT_out, make_lhsT):
                """hs_sb [128, NCH*HID] bf16 <- dinv*(prev @ W); then
                hT_out [64, DSH] bf16 <- relu(dinv_d * (hs.T @ A) + b)."""
                with tc.tile_pool(name="ph_psum", bufs=6, space="PSUM") as pp:
                    for c in range(NCH):
                        ph = pp.tile([128, HID], F32, tag="ph")
                        nc.tensor.matmul(ph[:], make_lhsT(c), W_t[:],
                                         start=True, stop=True)
                        nc.vector.tensor_scalar(
                            hs_sb[:, c * HID:(c + 1) * HID], ph[:],
                            dinvt[:, c:c + 1], None, ALU.mult)
                with tc.tile_pool(name="ag_psum", bufs=2, space="PSUM") as gp, \
                     tc.tile_pool(name="ag_tmp", bufs=2) as tp:
                    for h in range(2):
                        # Even chunks accumulate in PE column-group 0 (psum
                        # partitions 0:64), odd chunks in column-group 64 —
                        # the two matmul streams run concurrently.
                        pg = gp.tile([128, 512], F32, tag="pg")
                        for c in range(0, NCH, 2):
                            for u in range(2):
                                nc.tensor.matmul(
                                    pg[u * HID:(u + 1) * HID, :],
                                    hs_sb[:, (c + u) * HID:(c + u + 1) * HID],
                                    A_sb[:, (c + u) * DSH + h * 512:
                                         (c + u) * DSH + (h + 1) * 512],
                                    start=(c == 0), stop=(c == NCH - 2),
                                    tile_position=(0, u * HID),
                                    skip_group_check=True)
                        tmp = tp.tile([HID, 512], F32, tag="tmp")
                        tmp2 = tp.tile([HID, 512], F32, tag="tmp2")
                        dslice = dinvb[:, h * 512:(h + 1) * 512]
                        nc.vector.tensor_mul(tmp[:], pg[0:HID, :], dslice)
                        nc.vector.tensor_mul(tmp2[:], pg[HID:128, :], dslice)
                        nc.vector.tensor_add(tmp[:], tmp[:], tmp2[:])
                        nc.scalar.activation(hT_out[:, h * 512:(h + 1) * 512],
                                             tmp[:], AF.Relu, bias=b_t[:])

            def allgather_T(hT_sh, full_sb, idx):
                agin = dpool.tile([HID, DSH], BF16, name=f"agin{idx}")
                agout = dpool.tile([CORES * HID, DSH], BF16,
                                   addr_space="Shared", name=f"agout{idx}")
                nc.gpsimd.dma_start(agin[:], hT_sh[:])
                nc.gpsimd.collective_compute(
                    "AllGather", ALU.bypass,
                    replica_groups=[list(range(CORES))],
                    ins=[agin.opt()], outs=[agout.opt()])
                nc.sync.dma_start(
                    full_sb.rearrange("f (r j) -> f r j", j=DSH),
                    agout.rearrange("(r f) j -> f r j", f=HID))

            # ---- layer 1 ----
            with tc.tile_pool(name="l1", bufs=1) as l1pool:
                xT_sb = l1pool.tile([IN_DIM, N], BF16)
                for a in range(4):
                    sl = slice(a * (N // 4), (a + 1) * (N // 4))
                    nc.sync.dma_start(xT_sb[:, sl], xT_d[:, sl])
                hs1 = l1pool.tile([128, NCH * HID], BF16)
                gcn_layer(hs1, W1_t, b1_t, h1T_shard,
                          lambda c: xT_sb[:, c * 128:(c + 1) * 128])

            # ---- allgather h1, layer 2 ----
            with tc.tile_pool(name="l2", bufs=1) as l2pool:
                h1T_full = l2pool.tile([HID, N], BF16)
                allgather_T(h1T_shard, h1T_full, 1)
                hs2 = l2pool.tile([128, NCH * HID], BF16)
                gcn_layer(hs2, W2_t, b2_t, h2T_shard,
                          lambda c: h1T_full[:, c * 128:(c + 1) * 128])

            # ---- allgather h2, similarity + sigmoid ----
            with tc.tile_pool(name="sim", bufs=1) as spool, \
                 tc.tile_pool(name="sim_psum", bufs=3, space="PSUM") as sp, \
                 tc.tile_pool(name="stage", bufs=3) as stpool:
                h2T_full = spool.tile([HID, N], BF16)
                allgather_T(h2T_shard, h2T_full, 2)
                for m in range(DSH // 128):
                    lhsT = h2T_shard[:, m * 128:(m + 1) * 128]
                    for q in range(4):
                        st = stpool.tile([128, 2048], F32, tag="st")
                        for k in range(2):
                            ps = sp.tile([128, 1024], F32, tag="ps")
                            for u in range(2):
                                j = q * 2048 + k * 1024 + u * 512
                                nc.tensor.matmul(
                                    ps[:, u * 512:(u + 1) * 512], lhsT,
                                    h2T_full[:, j:j + 512],
                                    start=True, stop=True)
                            nc.scalar.activation(
                                st[:, k * 1024:(k + 1) * 1024], ps[:],
                                AF.Sigmoid)
                        nc.sync.dma_start(
                            out_d[m * 128:(m + 1) * 128,
                                  q * 2048:(q + 1) * 2048], st[:])

    nc.compile()
    return nc


def _get_program():
    if "nc" not in _COMPILED:
        _COMPILED["nc"] = _build_program()
    return _COMPILED["nc"]


def _prep_inputs(x, edge_index, W1, b1, W2, b2):
    x = np.asarray(x, np.float32)
    ei = np.asarray(edge_index)
    src = ei[0].astype(np.int64)
    dst = ei[1].astype(np.int64)

    deg = (np.bincount(dst, minlength=N) + 1).astype(np.float32)
    xT = np.ascontiguousarray(x.T).astype(ml_dtypes.bfloat16)      # [128, N]
    degt = np.ascontiguousarray(deg.reshape(NCH, 128).T)           # [128, 64]
    W1c = np.asarray(W1, np.float32).astype(ml_dtypes.bfloat16)
    W2c = np.asarray(W2, np.float32).astype(ml_dtypes.bfloat16)
    b1c = np.asarray(b1, np.float32).reshape(HID, 1).copy()
    b2c = np.asarray(b2, np.float32).reshape(HID, 1).copy()

    in_maps = []
    for i in range(CORES):
        lo = i * DSH
        sel = (dst >= lo) & (dst < lo + DSH)
        flat = src[sel] * DSH + (dst[sel] - lo)
        cnt = np.bincount(flat, minlength=N * DSH).reshape(N, DSH)
        cnt[np.arange(lo, lo + DSH), np.arange(DSH)] += 1          # + I shard
        # SBUF layout: partition p holds src rows {c*128+p}, free = c*DSH + d
        A8 = np.ascontiguousarray(
            cnt.reshape(NCH, 128, DSH).transpose(1, 0, 2)
        ).astype(ml_dtypes.float8_e4m3).reshape(128, NCH * DSH)
        degb = np.ascontiguousarray(
            np.broadcast_to(deg[lo:lo + DSH][None, :], (HID, DSH)))
        in_maps.append({
            "xT": xT, "A": A8, "degt": degt, "degb": degb,
            "W1": W1c, "W2": W2c, "b1": b1c, "b2": b2c,
        })
    return in_maps


def kernel(x, edge_index, W1, b1, W2, b2, _trace=False, _trace_kwargs=None):
    nc = _get_program()
    in_maps = _prep_inputs(x, edge_index, W1, b1, W2, b2)
    res = run_bass_kernel_spmd(nc, in_maps, core_ids=list(range(CORES)),
                               trace=_trace, **(_trace_kwargs or {}))
    out = np.concatenate([res.results[i]["out"] for i in range(CORES)], axis=0)
    if _trace:
        kernel._last_results = res
    return np.ascontiguousarray(out, dtype=np.float32)


# revision 7
# speedup vs baseline: 1.8249x; 1.0828x over previous
"""Trainium2 Bass kernel for a 2-layer GCN + sigmoid similarity matrix.

Model (see reference):
    h1 = relu(gcn_conv(x, W1, b1));  h2 = relu(gcn_conv(h1, W2, b2))
    out = sigmoid(h2 @ h2.T)                               # [8192, 8192]

gcn_conv(x, W, b) with self-loops and symmetric deg^{-1/2} norm factorizes:
    h  = x @ W
    out[d] = dinv[d] * sum_s Ahat[s, d] * (dinv[s] * h[s]) + b
where Ahat = edge-count matrix + I and dinv = rsqrt(indeg + 1).

Distribution over 8 NeuronCores (dst-sharded, per the sharding hint):
  - Every core computes hs1 = dinv * (x @ W1) for ALL nodes (cheap, replicated)
  - Ahat is densified per core as the [8192 src, 1024 dst] column shard, stored
    fp8_e4m3 (exact small integer counts) -> 8.4MB resident in SBUF.
  - Aggregation is a PE matmul: aggT[f, d] = sum_s hs[s, f] * Ahat[s, d],
    accumulated over 64 src chunks of 128 (lhsT = hs chunk bf16, rhs = A fp8),
    with even/odd chunks in separate PE column groups (concurrent matmuls).
  - Layer-1 output stays feature-major ([64, 1024] bf16 shard). Each core then
    computes hs2 = dinv * (h1 @ W2) for its own shard only and AllGathers the
    row-major [8192, 64] hs2 table, which feeds layer-2 aggregation directly.
  - h2 shards are AllGathered feature-major into [64, 8192]; each core computes
    its [1024, 8192] block of sigmoid(h2 @ h2.T) (bf16 PE matmul K=64 +
    ScalarE sigmoid from PSUM) and streams it out (DMA-bound at ~350 GB/s).

All TensorEngine operands are bf16/fp8 (fp32 matmul runs as two PE passes);
PSUM accumulation and the sigmoid are fp32.
"""

import os
import sys

# bass/concourse toolchain location (not a problem-statement file)
for _p in ("/opt/trn_rl_repo", "/root/.axon_site/_ro/trn_rl_repo"):
    if os.path.isdir(_p) and _p not in sys.path:
        sys.path.insert(0, _p)
        break

# A cpu-forced JAX would hide the axon-tunneled NeuronCores this kernel needs.
if os.environ.get("JAX_PLATFORMS", "").strip().lower() in ("cpu",):
    os.environ.pop("JAX_PLATFORMS")

import numpy as np
import ml_dtypes

import concourse.bass as bass
import concourse.bacc as bacc
import concourse.mybir as mybir
from concourse import tile
from concourse.bass_utils import run_bass_kernel_spmd

N = 8192          # nodes
E = 262144        # edges
IN_DIM = 128
HID = 64
CORES = 8
DSH = N // CORES  # dst shard size (1024)
NCH = N // 128    # src chunks of 128 (64)
KSH = DSH // 128  # chunks per shard (8)

F32 = mybir.dt.float32
BF16 = mybir.dt.bfloat16
FP8 = mybir.dt.float8e4
AF = mybir.ActivationFunctionType
ALU = mybir.AluOpType

_COMPILED = {}


def _build_program():
    nc = bacc.Bacc("TRN2", target_bir_lowering=False, debug=False,
                   num_devices=CORES)

    # ---- I/O ----
    xT_d = nc.dram_tensor("xT", [IN_DIM, N], BF16, kind="ExternalInput")
    A_d = nc.dram_tensor("A", [128, NCH * DSH], FP8, kind="ExternalInput")
    degt_d = nc.dram_tensor("degt", [128, NCH], F32, kind="ExternalInput")
    degb_d = nc.dram_tensor("degb", [HID, DSH], F32, kind="ExternalInput")
    degsh_d = nc.dram_tensor("degsh", [128, KSH], F32, kind="ExternalInput")
    W1_d = nc.dram_tensor("W1", [IN_DIM, HID], BF16, kind="ExternalInput")
    W2_d = nc.dram_tensor("W2", [HID, HID], BF16, kind="ExternalInput")
    b1_d = nc.dram_tensor("b1", [HID, 1], F32, kind="ExternalInput")
    b2_d = nc.dram_tensor("b2", [HID, 1], F32, kind="ExternalInput")
    out_d = nc.dram_tensor("out", [DSH, N], F32, kind="ExternalOutput")

    def rsqrt_newton(pool, deg_t, p, fd, tag):
        """dinv = rsqrt(deg): DVE reciprocal + ACT sqrt + one Newton step."""
        r = pool.tile([p, fd], F32, name=f"rs_r_{tag}")
        y = pool.tile([p, fd], F32, name=f"rs_y_{tag}")
        t = pool.tile([p, fd], F32, name=f"rs_t_{tag}")
        nc.vector.reciprocal(r[:], deg_t[:])
        nc.scalar.activation(y[:], r[:], AF.Sqrt)
        nc.vector.tensor_mul(t[:], y[:], y[:])
        nc.vector.tensor_mul(t[:], t[:], deg_t[:])
        nc.vector.tensor_scalar(t[:], t[:], -0.5, 1.5, ALU.mult, ALU.add)
        nc.vector.tensor_mul(y[:], y[:], t[:])
        return y

    with tile.TileContext(nc) as tc:
        with tc.tile_pool(name="const", bufs=1) as cpool, \
             tc.tile_pool(name="amat", bufs=1) as apool, \
             tc.tile_pool(name="dram", bufs=1, space="DRAM") as dpool:

            # ---- constants first (ACT HWDGE ring; doesn't queue behind the
            # bulk loads on the sync ring) ----
            W1_t = cpool.tile([IN_DIM, HID], BF16)
            W2_t = cpool.tile([HID, HID], BF16)
            b1_t = cpool.tile([HID, 1], F32)
            b2_t = cpool.tile([HID, 1], F32)
            degt_t = cpool.tile([128, NCH], F32)
            degb_t = cpool.tile([HID, DSH], F32)
            degsh_t = cpool.tile([128, KSH], F32)
            for t_, d_ in ((degt_t, degt_d), (W1_t, W1_d), (b1_t, b1_d),
                           (W2_t, W2_d), (b2_t, b2_d), (degsh_t, degsh_d),
                           (degb_t, degb_d)):
                nc.scalar.dma_start(t_[:], d_[:])

            # x (feature-major) on the ACT ring too, in pieces so layer-1
            # matmuls start as soon as the first piece lands.
            xT_sb = cpool.tile([IN_DIM, N], BF16)
            for a in range(8):
                sl = slice(a * (N // 8), (a + 1) * (N // 8))
                nc.scalar.dma_start(xT_sb[:, sl], xT_d[:, sl])

            # A (fp8, SBUF-resident) on the sync ring, in pieces.
            A_sb = apool.tile([128, NCH * DSH], FP8)
            APIECE = NCH // 8
            for a in range(8):
                sl = slice(a * APIECE * DSH, (a + 1) * APIECE * DSH)
                nc.sync.dma_start(A_sb[:, sl], A_d[:, sl])

            # Warm-up AllGather (same size-class as the real ones): absorbs
            # the collectives entry barrier + ncfw wakeup while inputs load.
            warm_sb = cpool.tile([HID, DSH], BF16)
            nc.gpsimd.memset(warm_sb[:], 0.0)
            warm_in = dpool.tile([HID, DSH], BF16)
            warm_out = dpool.tile([CORES * HID, DSH], BF16,
                                  addr_space="Shared")
            nc.gpsimd.dma_start(warm_in[:], warm_sb[:])
            nc.gpsimd.collective_compute(
                "AllGather", ALU.bypass,
                replica_groups=[list(range(CORES))],
                ins=[warm_in.opt()], outs=[warm_out.opt()])

            dinvt = rsqrt_newton(cpool, degt_t, 128, NCH, "t")   # [128, 64]
            dinvsh = rsqrt_newton(cpool, degsh_t, 128, KSH, "s")  # [128, 8]
            dinvb = rsqrt_newton(cpool, degb_t, HID, DSH, "b")   # [64, 1024]

            h1T_shard = cpool.tile([HID, DSH], BF16)
            h2T_shard = cpool.tile([HID, DSH], BF16)

            def linear_scaled(hs_sb, n_groups, make_lhsT, W_t, dinv_ap):
                """hs_sb[:, g*512:(g+1)*512] = dinv * (prev @ W) for
                n_groups groups of 8 chunks; dinv_ap [128, 8g] col-bcast."""
                with tc.tile_pool(name="ph_psum", bufs=3, space="PSUM") as pp:
                    for g in range(n_groups):
                        ph = pp.tile([128, 8 * HID], F32, tag="ph")
                        for k in range(8):
                            nc.tensor.matmul(
                                ph[:, k * HID:(k + 1) * HID],
                                make_lhsT(g * 8 + k), W_t[:],
                                start=True, stop=True)
                        dv = dinv_ap[:, g * 8:(g + 1) * 8]
                        nc.vector.tensor_tensor(
                            hs_sb.rearrange("p (c f) -> p c f", f=HID)
                                 [:, g * 8:(g + 1) * 8, :],
                            ph.rearrange("p (c f) -> p c f", f=HID),
                            dv.unsqueeze(2).broadcast_to((128, 8, HID)),
                            ALU.mult)

            def aggregate(hs_sb, b_t, hT_out):
                """hT_out [64, DSH] bf16 = relu(dinv_d * (hs.T @ A) + b),
                even/odd chunks in concurrent PE column groups."""
                with tc.tile_pool(name="ag_psum", bufs=2, space="PSUM") as gp, \
                     tc.tile_pool(name="ag_tmp", bufs=2) as tp:
                    for h in range(2):
                        pg = gp.tile([128, 512], F32, tag="pg")
                        for c in range(0, NCH, 2):
                            for u in range(2):
                                nc.tensor.matmul(
                                    pg[u * HID:(u + 1) * HID, :],
                                    hs_sb[:, (c + u) * HID:(c + u + 1) * HID],
                                    A_sb[:, (c + u) * DSH + h * 512:
                                         (c + u) * DSH + (h + 1) * 512],
                                    start=(c == 0), stop=(c == NCH - 2),
                                    tile_position=(0, u * HID),
                                    skip_group_check=True)
                        tmp = tp.tile([HID, 512], F32, tag="tmp")
                        tmp2 = tp.tile([HID, 512], F32, tag="tmp2")
                        dslice = dinvb[:, h * 512:(h + 1) * 512]
                        nc.vector.tensor_mul(tmp[:], pg[0:HID, :], dslice)
                        nc.vector.tensor_mul(tmp2[:], pg[HID:128, :], dslice)
                        nc.vector.tensor_add(tmp[:], tmp[:], tmp2[:])
                        nc.scalar.activation(hT_out[:, h * 512:(h + 1) * 512],
                                             tmp[:], AF.Relu, bias=b_t[:])

            # ---- layer 1: hs1 for ALL nodes (replicated), aggregate shard --
            with tc.tile_pool(name="l1", bufs=1) as l1pool:
                hs1 = l1pool.tile([128, NCH * HID], BF16)
                linear_scaled(hs1, NCH // 8,
                              lambda c: xT_sb[:, c * 128:(c + 1) * 128],
                              W1_t, dinvt)
                aggregate(hs1, b1_t, h1T_shard)

            # ---- hs2 for OWN shard, allgather row-major, layer 2 ----------
            with tc.tile_pool(name="l2", bufs=1) as l2pool:
                hs2_sh = l2pool.tile([128, KSH * HID], BF16)
                linear_scaled(hs2_sh, 1,
                              lambda k: h1T_shard[:, k * 128:(k + 1) * 128],
                              W2_t, dinvsh)
                ag2in = dpool.tile([DSH, HID], BF16, name="ag2in")
                ag2out = dpool.tile([N, HID], BF16, addr_space="Shared",
                                    name="ag2out")
                nc.gpsimd.dma_start(
                    ag2in.rearrange("(k p) f -> p k f", p=128),
                    hs2_sh.rearrange("p (k f) -> p k f", f=HID))
                nc.gpsimd.collective_compute(
                    "AllGather", ALU.bypass,
                    replica_groups=[list(range(CORES))],
                    ins=[ag2in.opt()], outs=[ag2out.opt()])
                hs2 = l2pool.tile([128, NCH * HID], BF16)
                nc.sync.dma_start(
                    hs2.rearrange("p (c f) -> p c f", f=HID),
                    ag2out.rearrange("(c p) f -> p c f", p=128))
                aggregate(hs2, b2_t, h2T_shard)

            # ---- allgather h2 feature-major, similarity + sigmoid ---------
            with tc.tile_pool(name="sim", bufs=1) as spool, \
                 tc.tile_pool(name="sim_psum", bufs=4, space="PSUM") as sp, \
                 tc.tile_pool(name="stage", bufs=3) as stpool:
                ag3in = dpool.tile([HID, DSH], BF16, name="ag3in")
                ag3out = dpool.tile([CORES * HID, DSH], BF16,
                                    addr_space="Shared", name="ag3out")
                nc.gpsimd.dma_start(ag3in[:], h2T_shard[:])
                nc.gpsimd.collective_compute(
                    "AllGather", ALU.bypass,
                    replica_groups=[list(range(CORES))],
                    ins=[ag3in.opt()], outs=[ag3out.opt()])
                h2T_full = spool.tile([HID, N], BF16)
                nc.sync.dma_start(
                    h2T_full.rearrange("f (r j) -> f r j", j=DSH),
                    ag3out.rearrange("(r f) j -> f r j", f=HID))

                for m in range(DSH // 128):
                    lhsT = h2T_shard[:, m * 128:(m + 1) * 128]
                    for q in range(4):
                        st = stpool.tile([128, 2048], F32, tag="st")
                        for k in range(2):
                            ps = sp.tile([128, 1024], F32, tag="ps")
                            for u in range(2):
                                j = q * 2048 + k * 1024 + u * 512
                                nc.tensor.matmul(
                                    ps[:, u * 512:(u + 1) * 512], lhsT,
                                    h2T_full[:, j:j + 512],
                                    start=True, stop=True)
                            nc.scalar.activation(
                                st[:, k * 1024:(k + 1) * 1024], ps[:],
                                AF.Sigmoid)
                        nc.sync.dma_start(
                            out_d[m * 128:(m + 1) * 128,
                                  q * 2048:(q + 1) * 2048], st[:])

    nc.compile()
    return nc


def _get_program():
    if "nc" not in _COMPILED:
        _COMPILED["nc"] = _build_program()
    return _COMPILED["nc"]


def _prep_inputs(x, edge_index, W1, b1, W2, b2):
    x = np.asarray(x, np.float32)
    ei = np.asarray(edge_index)
    src = ei[0].astype(np.int64)
    dst = ei[1].astype(np.int64)

    deg = (np.bincount(dst, minlength=N) + 1).astype(np.float32)
    xT = np.ascontiguousarray(x.T).astype(ml_dtypes.bfloat16)      # [128, N]
    degt = np.ascontiguousarray(deg.reshape(NCH, 128).T)           # [128, 64]
    W1c = np.asarray(W1, np.float32).astype(ml_dtypes.bfloat16)
    W2c = np.asarray(W2, np.float32).astype(ml_dtypes.bfloat16)
    b1c = np.asarray(b1, np.float32).reshape(HID, 1).copy()
    b2c = np.asarray(b2, np.float32).reshape(HID, 1).copy()

    in_maps = []
    for i in range(CORES):
        lo = i * DSH
        sel = (dst >= lo) & (dst < lo + DSH)
        flat = src[sel] * DSH + (dst[sel] - lo)
        cnt = np.bincount(flat, minlength=N * DSH).reshape(N, DSH)
        cnt[np.arange(lo, lo + DSH), np.arange(DSH)] += 1          # + I shard
        # SBUF layout: partition p holds src rows {c*128+p}, free = c*DSH + d
        A8 = np.ascontiguousarray(
            cnt.reshape(NCH, 128, DSH).transpose(1, 0, 2)
        ).astype(ml_dtypes.float8_e4m3).reshape(128, NCH * DSH)
        degb = np.ascontiguousarray(
            np.broadcast_to(deg[lo:lo + DSH][None, :], (HID, DSH)))
        degsh = np.ascontiguousarray(
            deg[lo:lo + DSH].reshape(KSH, 128).T)                  # [128, 8]
        in_maps.append({
            "xT": xT, "A": A8, "degt": degt, "degb": degb, "degsh": degsh,
            "W1": W1c, "W2": W2c, "b1": b1c, "b2": b2c,
        })
    return in_maps


def kernel(x, edge_index, W1, b1, W2, b2, _trace=False, _trace_kwargs=None):
    nc = _get_program()
    in_maps = _prep_inputs(x, edge_index, W1, b1, W2, b2)
    res = run_bass_kernel_spmd(nc, in_maps, core_ids=list(range(CORES)),
                               trace=_trace, **(_trace_kwargs or {}))
    out = np.concatenate([res.results[i]["out"] for i in range(CORES)], axis=0)
    if _trace:
        kernel._last_results = res
    return np.ascontiguousarray(out, dtype=np.float32)


# revision 8
# speedup vs baseline: 2.1158x; 1.1594x over previous
"""Trainium2 Bass kernel for a 2-layer GCN + sigmoid similarity matrix.

Model (see reference):
    h1 = relu(gcn_conv(x, W1, b1));  h2 = relu(gcn_conv(h1, W2, b2))
    out = sigmoid(h2 @ h2.T)                               # [8192, 8192]

gcn_conv(x, W, b) with self-loops and symmetric deg^{-1/2} norm factorizes:
    h  = x @ W
    out[d] = dinv[d] * sum_s Ahat[s, d] * (dinv[s] * h[s]) + b
where Ahat = edge-count matrix + I and dinv = rsqrt(indeg + 1).

Distribution over 8 NeuronCores (dst-sharded, per the sharding hint):
  - Every core computes hs1 = dinv * (x @ W1) for ALL nodes (cheap, replicated)
  - Ahat is densified per core as the [8192 src, 1024 dst] column shard, stored
    fp8_e4m3 (exact small integer counts) -> 8.4MB resident in SBUF.
  - Aggregation is a PE matmul: aggT[f, d] = sum_s hs[s, f] * Ahat[s, d],
    accumulated over 64 src chunks of 128 (lhsT = hs chunk bf16, rhs = A fp8),
    with even/odd chunks in separate PE column groups (concurrent matmuls).
  - Layer-1 output stays feature-major ([64, 1024] bf16 shard). Each core then
    computes hs2 = dinv * (h1 @ W2) for its own shard only and AllGathers the
    row-major [8192, 64] hs2 table, which feeds layer-2 aggregation directly.
  - h2 shards are AllGathered feature-major into [64, 8192]; each core computes
    its [1024, 8192] block of sigmoid(h2 @ h2.T) (bf16 PE matmul K=64 +
    ScalarE sigmoid from PSUM), written as bf16 and upcast to f32 on the host
    (sigmoid outputs here are ~0.52..0.60; bf16 costs ~1e-3 abs err).

Notes: all TensorEngine operands are bf16/fp8 (fp32 matmul runs as two PE
passes); a tiny first AllGather starts the collectives entry barrier (~45us)
early so it overlaps the input DMAs; constants ride one packed f32 tensor and
the weights ride in the x tensor so the startup issues few DMAs.
"""

import os
import sys

# bass/concourse toolchain location (not a problem-statement file)
for _p in ("/opt/trn_rl_repo", "/root/.axon_site/_ro/trn_rl_repo"):
    if os.path.isdir(_p) and _p not in sys.path:
        sys.path.insert(0, _p)
        break

# A cpu-forced JAX would hide the axon-tunneled NeuronCores this kernel needs.
if os.environ.get("JAX_PLATFORMS", "").strip().lower() in ("cpu",):
    os.environ.pop("JAX_PLATFORMS")

import numpy as np
import ml_dtypes

import concourse.bass as bass
import concourse.bacc as bacc
import concourse.mybir as mybir
from concourse import tile
from concourse.bass_utils import run_bass_kernel_spmd

N = 8192          # nodes
E = 262144        # edges
IN_DIM = 128
HID = 64
CORES = 8
DSH = N // CORES  # dst shard size (1024)
NCH = N // 128    # src chunks of 128 (64)
KSH = DSH // 128  # chunks per shard (8)

# packed f32 const tensor columns: degt | degsh | degb2 | b1 | b2
C_DEGT = 0
C_DEGSH = NCH                 # 64
C_DEGB = NCH + KSH            # 72
C_B1 = C_DEGB + 512           # 584
C_B2 = C_B1 + 1               # 585
C_COLS = C_B2 + 1             # 586
# packed bf16 tensor columns: W1 | W2(padded) | xT
XW_COLS = HID + HID + N

F32 = mybir.dt.float32
BF16 = mybir.dt.bfloat16
FP8 = mybir.dt.float8e4
AF = mybir.ActivationFunctionType
ALU = mybir.AluOpType

_COMPILED = {}


def _build_program():
    nc = bacc.Bacc("TRN2", target_bir_lowering=False, debug=False,
                   num_devices=CORES)

    # ---- I/O ----
    xw_d = nc.dram_tensor("xw", [128, XW_COLS], BF16, kind="ExternalInput")
    A_d = nc.dram_tensor("A", [128, NCH * DSH], FP8, kind="ExternalInput")
    c32_d = nc.dram_tensor("c32", [128, C_COLS], F32, kind="ExternalInput")
    out_d = nc.dram_tensor("out", [DSH, N], BF16, kind="ExternalOutput")

    with tile.TileContext(nc) as tc:
        with tc.tile_pool(name="const", bufs=1) as cpool, \
             tc.tile_pool(name="amat", bufs=1) as apool, \
             tc.tile_pool(name="dram", bufs=1, space="DRAM") as dpool:

            # Tiny first collective: starts the entry barrier + ncfw wakeup
            # immediately, overlapping the input DMAs. Must be cheap — it
            # serializes ahead of the first real AllGather on the CC stream.
            warm_sb = cpool.tile([64, 16], BF16)
            nc.gpsimd.memset(warm_sb[:], 0.0)
            warm_in = dpool.tile([64, 16], BF16)
            warm_out = dpool.tile([CORES * 64, 16], BF16, addr_space="Shared")
            nc.gpsimd.dma_start(warm_in[:], warm_sb[:])
            nc.gpsimd.collective_compute(
                "AllGather", ALU.bypass,
                replica_groups=[list(range(CORES))],
                ins=[warm_in.opt()], outs=[warm_out.opt()])

            # ---- packed constants (one DMA on the ACT HWDGE ring) ----
            c32 = cpool.tile([128, C_COLS], F32)
            nc.scalar.dma_start(c32[:], c32_d[:])
            b1_ap = c32[0:HID, C_B1:C_B1 + 1]
            b2_ap = c32[0:HID, C_B2:C_B2 + 1]

            # x + weights (bf16) on the ACT ring, first piece carries W1/W2.
            xw_sb = cpool.tile([128, XW_COLS], BF16)
            nc.scalar.dma_start(xw_sb[:, 0:1152], xw_d[:, 0:1152])
            for a in range(7):
                sl = slice(1152 + a * 1024, 1152 + (a + 1) * 1024)
                nc.scalar.dma_start(xw_sb[:, sl], xw_d[:, sl])
            W1_ap = xw_sb[:, 0:HID]
            W2_ap = xw_sb[0:HID, HID:2 * HID]

            def xT_chunk(c):
                return xw_sb[:, 2 * HID + c * 128: 2 * HID + (c + 1) * 128]

            # A (fp8, SBUF-resident) on the sync ring, in pieces.
            A_sb = apool.tile([128, NCH * DSH], FP8)
            APIECE = NCH // 8
            for a in range(8):
                sl = slice(a * APIECE * DSH, (a + 1) * APIECE * DSH)
                nc.sync.dma_start(A_sb[:, sl], A_d[:, sl])

            # ---- dinv = rsqrt(deg): fast reciprocal + sqrt + Newton step,
            # one fused pipeline over all packed deg columns ----
            DC = C_B1  # 584 deg columns
            deg_all = c32[:, 0:DC]
            r_ = cpool.tile([128, DC], F32)
            dinv = cpool.tile([128, DC], F32)
            t_ = cpool.tile([128, DC], F32)
            nc.vector.reciprocal_approx_fast(r_[:], deg_all)
            nc.scalar.activation(dinv[:], r_[:], AF.Sqrt)
            nc.vector.tensor_mul(t_[:], dinv[:], dinv[:])
            nc.vector.tensor_mul(t_[:], t_[:], deg_all)
            nc.vector.tensor_scalar(t_[:], t_[:], -0.5, 1.5, ALU.mult, ALU.add)
            nc.vector.tensor_mul(dinv[:], dinv[:], t_[:])
            dinvt = dinv[:, C_DEGT:C_DEGT + NCH]       # [128, 64]
            dinvsh = dinv[:, C_DEGSH:C_DEGSH + KSH]    # [128, 8]
            dinvb2 = dinv[:, C_DEGB:C_DEGB + 512]      # [128, 512] (2x64 halves)

            h1T_shard = cpool.tile([HID, DSH], BF16)
            h2T_shard = cpool.tile([HID, DSH], BF16)

            def linear_scaled(hs_sb, n_groups, make_lhsT, W_ap, dinv_ap):
                """hs_sb = dinv * (prev @ W), groups of 8 chunks per psum."""
                with tc.tile_pool(name="ph_psum", bufs=3, space="PSUM") as pp:
                    for g in range(n_groups):
                        ph = pp.tile([128, 8 * HID], F32, tag="ph")
                        for k in range(8):
                            nc.tensor.matmul(
                                ph[:, k * HID:(k + 1) * HID],
                                make_lhsT(g * 8 + k), W_ap,
                                start=True, stop=True)
                        dv = dinv_ap[:, g * 8:(g + 1) * 8]
                        nc.vector.tensor_tensor(
                            hs_sb.rearrange("p (c f) -> p c f", f=HID)
                                 [:, g * 8:(g + 1) * 8, :],
                            ph.rearrange("p (c f) -> p c f", f=HID),
                            dv.unsqueeze(2).broadcast_to((128, 8, HID)),
                            ALU.mult)

            def aggregate(hs_sb, b_ap, hT_out):
                """hT_out [64, DSH] bf16 = relu(dinv_d * (hs.T @ A) + b),
                even/odd chunks in concurrent PE column groups."""
                with tc.tile_pool(name="ag_psum", bufs=2, space="PSUM") as gp, \
                     tc.tile_pool(name="ag_tmp", bufs=2) as tp:
                    for h in range(2):
                        pg = gp.tile([128, 512], F32, tag="pg")
                        for c in range(0, NCH, 2):
                            for u in range(2):
                                nc.tensor.matmul(
                                    pg[u * HID:(u + 1) * HID, :],
                                    hs_sb[:, (c + u) * HID:(c + u + 1) * HID],
                                    A_sb[:, (c + u) * DSH + h * 512:
                                         (c + u) * DSH + (h + 1) * 512],
                                    start=(c == 0), stop=(c == NCH - 2),
                                    tile_position=(0, u * HID),
                                    skip_group_check=True)
                        tmp = tp.tile([HID, 512], F32, tag="tmp")
                        tmp2 = tp.tile([HID, 512], F32, tag="tmp2")
                        dslice = dinvb2[h * HID:(h + 1) * HID, :]
                        nc.vector.tensor_mul(tmp[:], pg[0:HID, :], dslice)
                        nc.vector.tensor_mul(tmp2[:], pg[HID:128, :], dslice)
                        nc.vector.tensor_add(tmp[:], tmp[:], tmp2[:])
                        nc.scalar.activation(hT_out[:, h * 512:(h + 1) * 512],
                                             tmp[:], AF.Relu, bias=b_ap)

            # ---- layer 1: hs1 for ALL nodes (replicated), aggregate shard --
            with tc.tile_pool(name="l1", bufs=1) as l1pool:
                hs1 = l1pool.tile([128, NCH * HID], BF16)
                linear_scaled(hs1, NCH // 8, xT_chunk, W1_ap, dinvt)
                aggregate(hs1, b1_ap, h1T_shard)

            # ---- hs2 for OWN shard, allgather row-major, layer 2 ----------
            with tc.tile_pool(name="l2", bufs=1) as l2pool:
                hs2_sh = l2pool.tile([128, KSH * HID], BF16)
                linear_scaled(hs2_sh, 1,
                              lambda k: h1T_shard[:, k * 128:(k + 1) * 128],
                              W2_ap, dinvsh)
                ag2in = dpool.tile([DSH, HID], BF16, name="ag2in")
                ag2out = dpool.tile([N, HID], BF16, addr_space="Shared",
                                    name="ag2out")
                nc.gpsimd.dma_start(
                    ag2in.rearrange("(k p) f -> p k f", p=128),
                    hs2_sh.rearrange("p (k f) -> p k f", f=HID))
                nc.gpsimd.collective_compute(
                    "AllGather", ALU.bypass,
                    replica_groups=[list(range(CORES))],
                    ins=[ag2in.opt()], outs=[ag2out.opt()])
                hs2 = l2pool.tile([128, NCH * HID], BF16)
                nc.sync.dma_start(
                    hs2.rearrange("p (c f) -> p c f", f=HID),
                    ag2out.rearrange("(c p) f -> p c f", p=128))
                aggregate(hs2, b2_ap, h2T_shard)

            # ---- allgather h2 feature-major, similarity + sigmoid ---------
            with tc.tile_pool(name="sim", bufs=1) as spool, \
                 tc.tile_pool(name="sim_psum", bufs=4, space="PSUM") as sp, \
                 tc.tile_pool(name="stage", bufs=3) as stpool:
                ag3in = dpool.tile([HID, DSH], BF16, name="ag3in")
                ag3out = dpool.tile([CORES * HID, DSH], BF16,
                                    addr_space="Shared", name="ag3out")
                nc.gpsimd.dma_start(ag3in[:], h2T_shard[:])
                nc.gpsimd.collective_compute(
                    "AllGather", ALU.bypass,
                    replica_groups=[list(range(CORES))],
                    ins=[ag3in.opt()], outs=[ag3out.opt()])
                h2T_full = spool.tile([HID, N], BF16)
                nc.sync.dma_start(
                    h2T_full.rearrange("f (r j) -> f r j", j=DSH),
                    ag3out.rearrange("(r f) j -> f r j", f=HID))

                for m in range(DSH // 128):
                    lhsT = h2T_shard[:, m * 128:(m + 1) * 128]
                    for q in range(4):
                        st = stpool.tile([128, 2048], BF16, tag="st")
                        for k in range(2):
                            ps = sp.tile([128, 1024], F32, tag="ps")
                            for u in range(2):
                                j = q * 2048 + k * 1024 + u * 512
                                nc.tensor.matmul(
                                    ps[:, u * 512:(u + 1) * 512], lhsT,
                                    h2T_full[:, j:j + 512],
                                    start=True, stop=True)
                            nc.scalar.activation(
                                st[:, k * 1024:(k + 1) * 1024], ps[:],
                                AF.Sigmoid)
                        nc.sync.dma_start(
                            out_d[m * 128:(m + 1) * 128,
                                  q * 2048:(q + 1) * 2048], st[:])

    nc.compile()
    return nc


def _get_program():
    if "nc" not in _COMPILED:
        _COMPILED["nc"] = _build_program()
    return _COMPILED["nc"]


def _prep_inputs(x, edge_index, W1, b1, W2, b2):
    x = np.asarray(x, np.float32)
    ei = np.asarray(edge_index)
    src = ei[0].astype(np.int64)
    dst = ei[1].astype(np.int64)

    deg = (np.bincount(dst, minlength=N) + 1).astype(np.float32)
    degt = np.ascontiguousarray(deg.reshape(NCH, 128).T)           # [128, 64]

    xw = np.zeros((128, XW_COLS), dtype=ml_dtypes.bfloat16)
    xw[:, 0:HID] = np.asarray(W1, np.float32).astype(ml_dtypes.bfloat16)
    xw[0:HID, HID:2 * HID] = (
        np.asarray(W2, np.float32).astype(ml_dtypes.bfloat16))
    xw[:, 2 * HID:] = x.T.astype(ml_dtypes.bfloat16)

    b1c = np.asarray(b1, np.float32).reshape(HID)
    b2c = np.asarray(b2, np.float32).reshape(HID)

    in_maps = []
    for i in range(CORES):
        lo = i * DSH
        sel = (dst >= lo) & (dst < lo + DSH)
        flat = src[sel] * DSH + (dst[sel] - lo)
        cnt = np.bincount(flat, minlength=N * DSH).reshape(N, DSH)
        cnt[np.arange(lo, lo + DSH), np.arange(DSH)] += 1          # + I shard
        # SBUF layout: partition p holds src rows {c*128+p}, free = c*DSH + d
        A8 = np.ascontiguousarray(
            cnt.reshape(NCH, 128, DSH).transpose(1, 0, 2)
        ).astype(ml_dtypes.float8_e4m3).reshape(128, NCH * DSH)

        c32 = np.zeros((128, C_COLS), dtype=np.float32)
        c32[:, C_DEGT:C_DEGT + NCH] = degt
        c32[:, C_DEGSH:C_DEGSH + KSH] = deg[lo:lo + DSH].reshape(KSH, 128).T
        degb = np.broadcast_to(deg[lo:lo + DSH][None, :], (HID, DSH))
        c32[:, C_DEGB:C_DEGB + 512] = (
            degb.reshape(HID, 2, 512).transpose(1, 0, 2).reshape(128, 512))
        c32[0:HID, C_B1] = b1c
        c32[0:HID, C_B2] = b2c
        # rsqrt pipeline runs over every deg column; keep the b columns out
        # of it but the whole c32 tile must be finite for the Newton step.
        in_maps.append({"xw": xw, "A": A8, "c32": c32})
    return in_maps


def kernel(x, edge_index, W1, b1, W2, b2, _trace=False, _trace_kwargs=None):
    nc = _get_program()
    in_maps = _prep_inputs(x, edge_index, W1, b1, W2, b2)
    res = run_bass_kernel_spmd(nc, in_maps, core_ids=list(range(CORES)),
                               trace=_trace, **(_trace_kwargs or {}))
    out = np.concatenate([res.results[i]["out"] for i in range(CORES)], axis=0)
    if _trace:
        kernel._last_results = res
    return out.astype(np.float32)


# revision 12
# speedup vs baseline: 2.1448x; 1.0137x over previous
"""Trainium2 Bass kernel for a 2-layer GCN + sigmoid similarity matrix.

Model (see reference):
    h1 = relu(gcn_conv(x, W1, b1));  h2 = relu(gcn_conv(h1, W2, b2))
    out = sigmoid(h2 @ h2.T)                               # [8192, 8192]

gcn_conv(x, W, b) with self-loops and symmetric deg^{-1/2} norm factorizes:
    h  = x @ W
    out[d] = dinv[d] * sum_s Ahat[s, d] * (dinv[s] * h[s]) + b
where Ahat = edge-count matrix + I and dinv = rsqrt(indeg + 1).

Distribution over 8 NeuronCores (dst-sharded, per the sharding hint):
  - Every core computes hs1 = dinv * (x @ W1) for ALL nodes (cheap, replicated)
  - Ahat is densified per core as the [8192 src, 1024 dst] column shard, stored
    fp8_e4m3 (exact small integer counts) -> 8.4MB resident in SBUF.
  - Aggregation is a PE matmul: aggT[f, d] = sum_s hs[s, f] * Ahat[s, d],
    accumulated over 64 src chunks of 128 (lhsT = hs chunk bf16, rhs = A fp8),
    with even/odd chunks in separate PE column groups (concurrent matmuls).
  - Layer-1 output stays feature-major ([64, 1024] bf16 shard). Each core then
    computes hs2 = dinv * (h1 @ W2) for its own shard only and AllGathers the
    row-major [8192, 64] hs2 table, which feeds layer-2 aggregation directly.
  - h2 shards are AllGathered feature-major into [64, 8192]; each core computes
    its [1024, 8192] block of sigmoid(h2 @ h2.T) (bf16 PE matmul K=64 +
    ScalarE sigmoid from PSUM), written as bf16 and upcast to f32 on the host
    (sigmoid outputs here are ~0.52..0.60; bf16 costs ~1e-3 abs err).

Notes: all TensorEngine operands are bf16/fp8 (fp32 matmul runs as two PE
passes); a tiny first AllGather starts the collectives entry barrier (~45us)
early so it overlaps the input DMAs; constants ride one packed f32 tensor and
the weights ride in the x tensor so the startup issues few DMAs.
"""

import os
import sys

# bass/concourse toolchain location (not a problem-statement file)
for _p in ("/opt/trn_rl_repo", "/root/.axon_site/_ro/trn_rl_repo"):
    if os.path.isdir(_p) and _p not in sys.path:
        sys.path.insert(0, _p)
        break

# A cpu-forced JAX would hide the axon-tunneled NeuronCores this kernel needs.
if os.environ.get("JAX_PLATFORMS", "").strip().lower() in ("cpu",):
    os.environ.pop("JAX_PLATFORMS")

import numpy as np
import ml_dtypes

import concourse.bass as bass
import concourse.bacc as bacc
import concourse.mybir as mybir
from concourse import tile
from concourse.bass_utils import run_bass_kernel_spmd

N = 8192          # nodes
E = 262144        # edges
IN_DIM = 128
HID = 64
CORES = 8
DSH = N // CORES  # dst shard size (1024)
NCH = N // 128    # src chunks of 128 (64)
KSH = DSH // 128  # chunks per shard (8)

# packed f32 const tensor columns: degt | degsh | degb2 | b1 | b2
C_DEGT = 0
C_DEGSH = NCH                 # 64
C_DEGB = NCH + KSH            # 72
C_B1 = C_DEGB + 512           # 584
C_B2 = C_B1 + 1               # 585
C_COLS = C_B2 + 1             # 586
# packed bf16 tensor columns: W1 | W2(padded) | xT
XW_COLS = HID + HID + N

F32 = mybir.dt.float32
BF16 = mybir.dt.bfloat16
FP8 = mybir.dt.float8e4
AF = mybir.ActivationFunctionType
ALU = mybir.AluOpType

_COMPILED = {}


def _build_program():
    nc = bacc.Bacc("TRN2", target_bir_lowering=False, debug=False,
                   num_devices=CORES)

    # ---- I/O ----
    xw_d = nc.dram_tensor("xw", [128, XW_COLS], BF16, kind="ExternalInput")
    A_d = nc.dram_tensor("A", [128, NCH * DSH], FP8, kind="ExternalInput")
    c32_d = nc.dram_tensor("c32", [128, C_COLS], F32, kind="ExternalInput")
    out_d = nc.dram_tensor("out", [DSH, N], BF16, kind="ExternalOutput")

    with tile.TileContext(nc) as tc:
        with tc.tile_pool(name="const", bufs=1) as cpool, \
             tc.tile_pool(name="amat", bufs=1) as apool, \
             tc.tile_pool(name="dram", bufs=1, space="DRAM") as dpool:

            # Tiny first collective: starts the entry barrier + ncfw wakeup
            # immediately, overlapping the input DMAs. Must be cheap — it
            # serializes ahead of the first real AllGather on the CC stream.
            warm_sb = cpool.tile([64, 16], BF16)
            nc.gpsimd.memset(warm_sb[:], 0.0)
            warm_in = dpool.tile([64, 16], BF16)
            warm_out = dpool.tile([CORES * 64, 16], BF16, addr_space="Shared")
            nc.gpsimd.dma_start(warm_in[:], warm_sb[:])
            nc.gpsimd.collective_compute(
                "AllGather", ALU.bypass,
                replica_groups=[list(range(CORES))],
                ins=[warm_in.opt()], outs=[warm_out.opt()])

            # ---- packed constants (one DMA on the ACT HWDGE ring) ----
            c32 = cpool.tile([128, C_COLS], F32)
            nc.scalar.dma_start(c32[:], c32_d[:])
            b1_ap = c32[0:HID, C_B1:C_B1 + 1]
            b2_ap = c32[0:HID, C_B2:C_B2 + 1]

            # x + weights (bf16) on the ACT ring, first piece carries W1/W2.
            xw_sb = cpool.tile([128, XW_COLS], BF16)
            nc.scalar.dma_start(xw_sb[:, 0:1152], xw_d[:, 0:1152])
            for a in range(7):
                sl = slice(1152 + a * 1024, 1152 + (a + 1) * 1024)
                nc.scalar.dma_start(xw_sb[:, sl], xw_d[:, sl])
            W1_ap = xw_sb[:, 0:HID]
            W2_ap = xw_sb[0:HID, HID:2 * HID]

            def xT_chunk(c):
                return xw_sb[:, 2 * HID + c * 128: 2 * HID + (c + 1) * 128]

            # A (fp8, SBUF-resident) on the sync ring, in pieces.
            A_sb = apool.tile([128, NCH * DSH], FP8)
            APIECE = NCH // 8
            for a in range(8):
                sl = slice(a * APIECE * DSH, (a + 1) * APIECE * DSH)
                nc.sync.dma_start(A_sb[:, sl], A_d[:, sl])

            # ---- dinv = rsqrt(deg): fast reciprocal + sqrt + Newton step,
            # one fused pipeline over all packed deg columns ----
            DC = C_B1  # 584 deg columns
            deg_all = c32[:, 0:DC]
            r_ = cpool.tile([128, DC], F32)
            dinv = cpool.tile([128, DC], F32)
            t_ = cpool.tile([128, DC], F32)
            nc.vector.reciprocal_approx_fast(r_[:], deg_all)
            nc.scalar.activation(dinv[:], r_[:], AF.Sqrt)
            nc.vector.tensor_mul(t_[:], dinv[:], dinv[:])
            nc.vector.tensor_mul(t_[:], t_[:], deg_all)
            nc.vector.tensor_scalar(t_[:], t_[:], -0.5, 1.5, ALU.mult, ALU.add)
            nc.vector.tensor_mul(dinv[:], dinv[:], t_[:])
            dinvt = dinv[:, C_DEGT:C_DEGT + NCH]       # [128, 64]
            dinvsh = dinv[:, C_DEGSH:C_DEGSH + KSH]    # [128, 8]
            dinvb2 = dinv[:, C_DEGB:C_DEGB + 512]      # [128, 512] (2x64 halves)

            h1T_shard = cpool.tile([HID, DSH], BF16)
            h2T_shard = cpool.tile([HID, DSH], BF16)

            def linear_scaled(hs_sb, n_groups, make_lhsT, W_ap, dinv_ap):
                """hs_sb = dinv * (prev @ W), groups of 8 chunks per psum."""
                with tc.tile_pool(name="ph_psum", bufs=3, space="PSUM") as pp:
                    for g in range(n_groups):
                        ph = pp.tile([128, 8 * HID], F32, tag="ph")
                        for k in range(8):
                            nc.tensor.matmul(
                                ph[:, k * HID:(k + 1) * HID],
                                make_lhsT(g * 8 + k), W_ap,
                                start=True, stop=True)
                        dv = dinv_ap[:, g * 8:(g + 1) * 8]
                        nc.vector.tensor_tensor(
                            hs_sb.rearrange("p (c f) -> p c f", f=HID)
                                 [:, g * 8:(g + 1) * 8, :],
                            ph.rearrange("p (c f) -> p c f", f=HID),
                            dv.unsqueeze(2).broadcast_to((128, 8, HID)),
                            ALU.mult)

            def aggregate(hs_sb, b_ap, hT_out):
                """hT_out [64, DSH] bf16 = relu(dinv_d * (hs.T @ A) + b).
                The two dst halves accumulate concurrently in separate PE
                column groups (same stationary hs chunk loaded to both), so
                both finish as soon as the last A chunk is consumed."""
                with tc.tile_pool(name="ag_psum", bufs=1, space="PSUM") as gp, \
                     tc.tile_pool(name="ag_tmp", bufs=2) as tp:
                    pg = gp.tile([128, 512], F32, tag="pg")
                    for c in range(NCH):
                        for h in range(2):
                            nc.tensor.matmul(
                                pg[h * HID:(h + 1) * HID, :],
                                hs_sb[:, c * HID:(c + 1) * HID],
                                A_sb[:, c * DSH + h * 512:
                                     c * DSH + (h + 1) * 512],
                                start=(c == 0), stop=(c == NCH - 1),
                                tile_position=(0, h * HID),
                                skip_group_check=True)
                    for h in range(2):
                        tmp = tp.tile([HID, 512], F32, tag="tmp")
                        nc.vector.tensor_mul(tmp[:], pg[h * HID:(h + 1) * HID, :],
                                             dinvb2[h * HID:(h + 1) * HID, :])
                        nc.scalar.activation(hT_out[:, h * 512:(h + 1) * 512],
                                             tmp[:], AF.Relu, bias=b_ap)

            # ---- layer 1: hs1 for ALL nodes (replicated), aggregate shard --
            with tc.tile_pool(name="l1", bufs=1) as l1pool:
                hs1 = l1pool.tile([128, NCH * HID], BF16)
                linear_scaled(hs1, NCH // 8, xT_chunk, W1_ap, dinvt)
                aggregate(hs1, b1_ap, h1T_shard)

            # ---- hs2 for OWN shard, allgather row-major, layer 2 ----------
            with tc.tile_pool(name="l2", bufs=1) as l2pool:
                hs2_sh = l2pool.tile([128, KSH * HID], BF16)
                linear_scaled(hs2_sh, 1,
                              lambda k: h1T_shard[:, k * 128:(k + 1) * 128],
                              W2_ap, dinvsh)
                ag2in = dpool.tile([DSH, HID], BF16, name="ag2in")
                ag2out = dpool.tile([N, HID], BF16, addr_space="Shared",
                                    name="ag2out")
                nc.gpsimd.dma_start(
                    ag2in.rearrange("(k p) f -> p k f", p=128),
                    hs2_sh.rearrange("p (k f) -> p k f", f=HID))
                nc.gpsimd.collective_compute(
                    "AllGather", ALU.bypass,
                    replica_groups=[list(range(CORES))],
                    ins=[ag2in.opt()], outs=[ag2out.opt()])
                hs2 = l2pool.tile([128, NCH * HID], BF16)
                for hh in range(2):
                    cs = slice(hh * (NCH // 2) * HID, (hh + 1) * (NCH // 2) * HID)
                    rs = slice(hh * (N // 2), (hh + 1) * (N // 2))
                    nc.sync.dma_start(
                        hs2[:, cs].rearrange("p (c f) -> p c f", f=HID),
                        ag2out[rs, :].rearrange("(c p) f -> p c f", p=128))
                aggregate(hs2, b2_ap, h2T_shard)

            # ---- allgather h2 feature-major, similarity + sigmoid ---------
            with tc.tile_pool(name="sim", bufs=1) as spool, \
                 tc.tile_pool(name="sim_psum", bufs=2, space="PSUM") as sp, \
                 tc.tile_pool(name="stage", bufs=3) as stpool:
                ag3in = dpool.tile([HID, DSH], BF16, name="ag3in")
                ag3out = dpool.tile([CORES * HID, DSH], BF16,
                                    addr_space="Shared", name="ag3out")
                nc.gpsimd.dma_start(ag3in[:], h2T_shard[:])
                nc.gpsimd.collective_compute(
                    "AllGather", ALU.bypass,
                    replica_groups=[list(range(CORES))],
                    ins=[ag3in.opt()], outs=[ag3out.opt()])
                h2T_full = spool.tile([HID, N], BF16)
                nc.sync.dma_start(
                    h2T_full.rearrange("f (r j) -> f r j", j=DSH),
                    ag3out.rearrange("(r f) j -> f r j", f=HID))

                for m in range(DSH // 128):
                    lhsT = h2T_shard[:, m * 128:(m + 1) * 128]
                    for q in range(4):
                        st = stpool.tile([128, 2048], BF16, tag="st")
                        ps = sp.tile([128, 2048], F32, tag="ps")
                        for u in range(4):
                            j = q * 2048 + u * 512
                            nc.tensor.matmul(
                                ps[:, u * 512:(u + 1) * 512], lhsT,
                                h2T_full[:, j:j + 512],
                                start=True, stop=True)
                        nc.scalar.activation(st[:], ps[:], AF.Sigmoid)
                        nc.sync.dma_start(
                            out_d[m * 128:(m + 1) * 128,
                                  q * 2048:(q + 1) * 2048], st[:])

    nc.compile()
    return nc


def _get_program():
    if "nc" not in _COMPILED:
        _COMPILED["nc"] = _build_program()
    return _COMPILED["nc"]


def _prep_inputs(x, edge_index, W1, b1, W2, b2):
    x = np.asarray(x, np.float32)
    ei = np.asarray(edge_index)
    src = ei[0].astype(np.int64)
    dst = ei[1].astype(np.int64)

    deg = (np.bincount(dst, minlength=N) + 1).astype(np.float32)
    degt = np.ascontiguousarray(deg.reshape(NCH, 128).T)           # [128, 64]

    xw = np.zeros((128, XW_COLS), dtype=ml_dtypes.bfloat16)
    xw[:, 0:HID] = np.asarray(W1, np.float32).astype(ml_dtypes.bfloat16)
    xw[0:HID, HID:2 * HID] = (
        np.asarray(W2, np.float32).astype(ml_dtypes.bfloat16))
    xw[:, 2 * HID:] = x.T.astype(ml_dtypes.bfloat16)

    b1c = np.asarray(b1, np.float32).reshape(HID)
    b2c = np.asarray(b2, np.float32).reshape(HID)

    in_maps = []
    for i in range(CORES):
        lo = i * DSH
        sel = (dst >= lo) & (dst < lo + DSH)
        flat = src[sel] * DSH + (dst[sel] - lo)
        cnt = np.bincount(flat, minlength=N * DSH).reshape(N, DSH)
        cnt[np.arange(lo, lo + DSH), np.arange(DSH)] += 1          # + I shard
        # SBUF layout: partition p holds src rows {c*128+p}, free = c*DSH + d
        A8 = np.ascontiguousarray(
            cnt.reshape(NCH, 128, DSH).transpose(1, 0, 2)
        ).astype(ml_dtypes.float8_e4m3).reshape(128, NCH * DSH)

        c32 = np.zeros((128, C_COLS), dtype=np.float32)
        c32[:, C_DEGT:C_DEGT + NCH] = degt
        c32[:, C_DEGSH:C_DEGSH + KSH] = deg[lo:lo + DSH].reshape(KSH, 128).T
        degb = np.broadcast_to(deg[lo:lo + DSH][None, :], (HID, DSH))
        c32[:, C_DEGB:C_DEGB + 512] = (
            degb.reshape(HID, 2, 512).transpose(1, 0, 2).reshape(128, 512))
        c32[0:HID, C_B1] = b1c
        c32[0:HID, C_B2] = b2c
        # rsqrt pipeline runs over every deg column; keep the b columns out
        # of it but the whole c32 tile must be finite for the Newton step.
        in_maps.append({"xw": xw, "A": A8, "c32": c32})
    return in_maps


def kernel(x, edge_index, W1, b1, W2, b2, _trace=False, _trace_kwargs=None):
    nc = _get_program()
    in_maps = _prep_inputs(x, edge_index, W1, b1, W2, b2)
    res = run_bass_kernel_spmd(nc, in_maps, core_ids=list(range(CORES)),
                               trace=_trace, **(_trace_kwargs or {}))
    out = np.concatenate([res.results[i]["out"] for i in range(CORES)], axis=0)
    if _trace:
        kernel._last_results = res
    return out.astype(np.float32)


# revision 17
# speedup vs baseline: 2.2030x; 1.0271x over previous
"""Trainium2 Bass kernel for a 2-layer GCN + sigmoid similarity matrix.

Model (see reference):
    h1 = relu(gcn_conv(x, W1, b1));  h2 = relu(gcn_conv(h1, W2, b2))
    out = sigmoid(h2 @ h2.T)                               # [8192, 8192]

gcn_conv(x, W, b) with self-loops and symmetric deg^{-1/2} norm factorizes:
    h  = x @ W
    out[d] = dinv[d] * sum_s Ahat[s, d] * (dinv[s] * h[s]) + b
where Ahat = edge-count matrix + I and dinv = rsqrt(indeg + 1).

Distribution over 8 NeuronCores (dst-sharded, per the sharding hint):
  - Every core computes hs1 = dinv * (x @ W1) for ALL nodes (cheap, replicated)
  - Ahat is densified per core as the [8192 src, 1024 dst] column shard, stored
    fp8_e4m3 (exact small integer counts) -> 8.4MB resident in SBUF.
  - Aggregation is a PE matmul: aggT[f, d] = sum_s hs[s, f] * Ahat[s, d],
    accumulated over 64 src chunks of 128 (lhsT = hs chunk bf16, rhs = A fp8),
    with even/odd chunks in separate PE column groups (concurrent matmuls).
  - Layer-1 output stays feature-major ([64, 1024] bf16 shard). Each core then
    computes hs2 = dinv * (h1 @ W2) for its own shard only and AllGathers the
    row-major [8192, 64] hs2 table, which feeds layer-2 aggregation directly.
  - h2 shards are AllGathered feature-major into [64, 8192]; each core computes
    its [1024, 8192] block of sigmoid(h2 @ h2.T) (bf16 PE matmul K=64 +
    ScalarE sigmoid from PSUM), written as bf16 and upcast to f32 on the host
    (sigmoid outputs here are ~0.52..0.60; bf16 costs ~1e-3 abs err).

Notes: all TensorEngine operands are bf16/fp8 (fp32 matmul runs as two PE
passes); a tiny first AllGather starts the collectives entry barrier (~45us)
early so it overlaps the input DMAs; constants ride one packed f32 tensor and
the weights ride in the x tensor so the startup issues few DMAs.
"""

import os
import sys

# bass/concourse toolchain location (not a problem-statement file)
for _p in ("/opt/trn_rl_repo", "/root/.axon_site/_ro/trn_rl_repo"):
    if os.path.isdir(_p) and _p not in sys.path:
        sys.path.insert(0, _p)
        break

# A cpu-forced JAX would hide the axon-tunneled NeuronCores this kernel needs.
if os.environ.get("JAX_PLATFORMS", "").strip().lower() in ("cpu",):
    os.environ.pop("JAX_PLATFORMS")

import numpy as np
import ml_dtypes

import concourse.bass as bass
import concourse.bacc as bacc
import concourse.mybir as mybir
from concourse import tile
from concourse.bass_utils import run_bass_kernel_spmd

N = 8192          # nodes
E = 262144        # edges
IN_DIM = 128
HID = 64
CORES = 8
DSH = N // CORES  # dst shard size (1024)
NCH = N // 128    # src chunks of 128 (64)
KSH = DSH // 128  # chunks per shard (8)

# packed f32 const tensor columns: degt | degsh | degb2 | b1 | b2
C_DEGT = 0
C_DEGSH = NCH                 # 64
C_DEGB = NCH + KSH            # 72
C_B1 = C_DEGB + 512           # 584
C_B2 = C_B1 + 1               # 585
C_COLS = C_B2 + 1             # 586
# packed bf16 tensor columns: W1 | W2(padded) | xT
XW_COLS = HID + HID + N

F32 = mybir.dt.float32
BF16 = mybir.dt.bfloat16
FP8 = mybir.dt.float8e4
AF = mybir.ActivationFunctionType
ALU = mybir.AluOpType

_COMPILED = {}


def _build_program():
    nc = bacc.Bacc("TRN2", target_bir_lowering=False, debug=False,
                   num_devices=CORES)

    # ---- I/O ----
    xw_d = nc.dram_tensor("xw", [128, XW_COLS], BF16, kind="ExternalInput")
    A_d = nc.dram_tensor("A", [128, NCH * DSH], FP8, kind="ExternalInput")
    c32_d = nc.dram_tensor("c32", [128, C_COLS], F32, kind="ExternalInput")
    out_d = nc.dram_tensor("out", [DSH, N], BF16, kind="ExternalOutput")

    with tile.TileContext(nc) as tc:
        with tc.tile_pool(name="const", bufs=1) as cpool, \
             tc.tile_pool(name="amat", bufs=1) as apool, \
             tc.tile_pool(name="dram", bufs=1, space="DRAM") as dpool:

            # Tiny first collective: starts the entry barrier + ncfw wakeup
            # immediately, overlapping the input DMAs. Must be cheap — it
            # serializes ahead of the first real AllGather on the CC stream.
            warm_sb = cpool.tile([64, 16], BF16)
            nc.gpsimd.memset(warm_sb[:], 0.0)
            warm_in = dpool.tile([64, 16], BF16)
            warm_out = dpool.tile([CORES * 64, 16], BF16, addr_space="Shared")
            nc.gpsimd.dma_start(warm_in[:], warm_sb[:])
            nc.gpsimd.collective_compute(
                "AllGather", ALU.bypass,
                replica_groups=[list(range(CORES))],
                ins=[warm_in.opt()], outs=[warm_out.opt()])

            # ---- packed constants (one DMA on the ACT HWDGE ring) ----
            c32 = cpool.tile([128, C_COLS], F32)
            nc.scalar.dma_start(c32[:], c32_d[:])
            b1_ap = c32[0:HID, C_B1:C_B1 + 1]
            b2_ap = c32[0:HID, C_B2:C_B2 + 1]

            # x + weights (bf16) on the ACT ring, first piece carries W1/W2.
            xw_sb = cpool.tile([128, XW_COLS], BF16)
            nc.scalar.dma_start(xw_sb[:, 0:1152], xw_d[:, 0:1152])
            for a in range(7):
                sl = slice(1152 + a * 1024, 1152 + (a + 1) * 1024)
                nc.scalar.dma_start(xw_sb[:, sl], xw_d[:, sl])
            W1_ap = xw_sb[:, 0:HID]
            W2_ap = xw_sb[0:HID, HID:2 * HID]

            def xT_chunk(c):
                return xw_sb[:, 2 * HID + c * 128: 2 * HID + (c + 1) * 128]

            # A (fp8, SBUF-resident) on the sync ring, in pieces.
            A_sb = apool.tile([128, NCH * DSH], FP8)
            APIECE = NCH // 8
            for a in range(8):
                sl = slice(a * APIECE * DSH, (a + 1) * APIECE * DSH)
                nc.sync.dma_start(A_sb[:, sl], A_d[:, sl])

            # ---- dinv = rsqrt(deg): fast reciprocal + sqrt + Newton step,
            # one fused pipeline over all packed deg columns ----
            DC = C_B1  # 584 deg columns
            deg_all = c32[:, 0:DC]
            r_ = cpool.tile([128, DC], F32)
            dinv = cpool.tile([128, DC], F32)
            t_ = cpool.tile([128, DC], F32)
            nc.vector.reciprocal_approx_fast(r_[:], deg_all)
            nc.scalar.activation(dinv[:], r_[:], AF.Sqrt)
            nc.vector.tensor_mul(t_[:], dinv[:], dinv[:])
            nc.vector.tensor_mul(t_[:], t_[:], deg_all)
            nc.vector.tensor_scalar(t_[:], t_[:], -0.5, 1.5, ALU.mult, ALU.add)
            nc.vector.tensor_mul(dinv[:], dinv[:], t_[:])
            dinvt = dinv[:, C_DEGT:C_DEGT + NCH]       # [128, 64]
            dinvsh = dinv[:, C_DEGSH:C_DEGSH + KSH]    # [128, 8]
            dinvb2 = dinv[:, C_DEGB:C_DEGB + 512]      # [128, 512] (2x64 halves)

            h1T_shard = cpool.tile([HID, DSH], BF16)
            h2T_shard = cpool.tile([HID, DSH], BF16)

            def linear_scaled(hs_sb, n_groups, make_lhsT, W_ap, dinv_ap):
                """hs_sb = dinv * (prev @ W), groups of 8 chunks per psum."""
                with tc.tile_pool(name="ph_psum", bufs=3, space="PSUM") as pp:
                    for g in range(n_groups):
                        ph = pp.tile([128, 8 * HID], F32, tag="ph")
                        for k in range(8):
                            nc.tensor.matmul(
                                ph[:, k * HID:(k + 1) * HID],
                                make_lhsT(g * 8 + k), W_ap,
                                start=True, stop=True)
                        dv = dinv_ap[:, g * 8:(g + 1) * 8]
                        nc.vector.tensor_tensor(
                            hs_sb.rearrange("p (c f) -> p c f", f=HID)
                                 [:, g * 8:(g + 1) * 8, :],
                            ph.rearrange("p (c f) -> p c f", f=HID),
                            dv.unsqueeze(2).broadcast_to((128, 8, HID)),
                            ALU.mult)

            def aggregate(hs_sb, b_ap, hT_out, order=None):
                """hT_out [64, DSH] bf16 = relu(dinv_d * (hs.T @ A) + b).
                The two dst halves accumulate concurrently in separate PE
                column groups (same stationary hs chunk loaded to both), so
                both finish as soon as the last chunk is consumed. `order`
                permutes the (sum-commutative) chunk visit order so chunks
                arriving from a split AllGather can be consumed first."""
                pairs = ([(c, c) for c in range(NCH)] if order is None
                         else order)  # (slot in hs_sb, chunk in A)
                with tc.tile_pool(name="ag_psum", bufs=1, space="PSUM") as gp, \
                     tc.tile_pool(name="ag_tmp", bufs=2) as tp:
                    pg = gp.tile([128, 512], F32, tag="pg")
                    for ci, (s, c) in enumerate(pairs):
                        for h in range(2):
                            nc.tensor.matmul(
                                pg[h * HID:(h + 1) * HID, :],
                                hs_sb[:, s * HID:(s + 1) * HID],
                                A_sb[:, c * DSH + h * 512:
                                     c * DSH + (h + 1) * 512],
                                start=(ci == 0), stop=(ci == NCH - 1),
                                tile_position=(0, h * HID),
                                skip_group_check=True)
                    for h in range(2):
                        tmp = tp.tile([HID, 512], F32, tag="tmp")
                        nc.vector.tensor_mul(tmp[:], pg[h * HID:(h + 1) * HID, :],
                                             dinvb2[h * HID:(h + 1) * HID, :])
                        nc.scalar.activation(hT_out[:, h * 512:(h + 1) * 512],
                                             tmp[:], AF.Relu, bias=b_ap)

            # ---- layer 1: hs1 for ALL nodes (replicated), aggregate shard --
            with tc.tile_pool(name="l1", bufs=1) as l1pool:
                hs1 = l1pool.tile([128, NCH * HID], BF16)
                linear_scaled(hs1, NCH // 8, xT_chunk, W1_ap, dinvt)
                aggregate(hs1, b1_ap, h1T_shard)

            # ---- hs2 for OWN shard, allgather row-major, layer 2 ----------
            with tc.tile_pool(name="l2", bufs=1) as l2pool:
                hs2_sh = l2pool.tile([128, KSH * HID], BF16)
                linear_scaled(hs2_sh, 1,
                              lambda k: h1T_shard[:, k * 128:(k + 1) * 128],
                              W2_ap, dinvsh)
                # AllGather hs2 in two halves (first/last 4 chunks of each
                # shard) so layer-2 aggregation starts on the first half
                # while the second is still in flight. hs2 slots are stored
                # in AG arrival order: slot = hh*32 + r*4 + k for node chunk
                # c = r*8 + hh*4 + k.
                hs2 = l2pool.tile([128, NCH * HID], BF16)
                for hh in range(2):
                    agin = dpool.tile([DSH // 2, HID], BF16, name=f"ag2in{hh}")
                    agout = dpool.tile([N // 2, HID], BF16,
                                       addr_space="Shared", name=f"ag2out{hh}")
                    nc.gpsimd.dma_start(
                        agin.rearrange("(k p) f -> p k f", p=128),
                        hs2_sh.rearrange("p (k f) -> p k f", f=HID)
                             [:, hh * 4:(hh + 1) * 4, :])
                    nc.gpsimd.collective_compute(
                        "AllGather", ALU.bypass,
                        replica_groups=[list(range(CORES))],
                        ins=[agin.opt()], outs=[agout.opt()])
                    nc.sync.dma_start(
                        hs2[:, hh * 2048:(hh + 1) * 2048]
                            .rearrange("p (q f) -> p q f", f=HID),
                        agout.rearrange("(q p) f -> p q f", p=128))
                order = [(hh * 32 + r * 4 + k, r * 8 + hh * 4 + k)
                         for hh in range(2) for r in range(CORES)
                         for k in range(4)]
                aggregate(hs2, b2_ap, h2T_shard, order=order)

            # ---- allgather h2 feature-major (two halves), sim + sigmoid ---
            # h2T is duplicated onto partitions 64:128 so pairs of j-tiles
            # run as concurrent K=64 matmuls in separate PE row groups.
            with tc.tile_pool(name="sim", bufs=1) as spool, \
                 tc.tile_pool(name="sim_psum", bufs=2, space="PSUM") as sp, \
                 tc.tile_pool(name="stage", bufs=3) as stpool:
                sh_dup = spool.tile([128, DSH], BF16)
                nc.sync.dma_start(sh_dup[0:HID, :], h2T_shard[:])
                nc.sync.dma_start(sh_dup[HID:128, :], h2T_shard[:])
                # free-dim layout of h2T_dup: pass p block at p*4096, then
                # rank r strip of 512 (= h2 cols r*1024 + p*512 + [0, 512))
                h2T_dup = spool.tile([128, N], BF16)
                for p in range(2):
                    agin = dpool.tile([HID, 512], BF16, name=f"ag3in{p}")
                    agout = dpool.tile([CORES * HID, 512], BF16,
                                       addr_space="Shared", name=f"ag3out{p}")
                    nc.gpsimd.dma_start(agin[:],
                                        h2T_shard[:, p * 512:(p + 1) * 512])
                    nc.gpsimd.collective_compute(
                        "AllGather", ALU.bypass,
                        replica_groups=[list(range(CORES))],
                        ins=[agin.opt()], outs=[agout.opt()])
                    src = agout.rearrange("(r f) j -> f r j", f=HID)
                    blk = h2T_dup[:, p * 4096:(p + 1) * 4096]
                    nc.sync.dma_start(
                        blk[0:HID, :].rearrange("f (r j) -> f r j", j=512), src)
                    nc.sync.dma_start(
                        blk[HID:128, :].rearrange("f (r j) -> f r j", j=512),
                        src)

                out4 = out_d.rearrange("m (r p j) -> m r p j", p=2, j=512)
                for p in range(2):
                    for m in range(DSH // 128):
                        for rq in range(2):
                            st = stpool.tile([128, 2048], BF16, tag="st")
                            ps = sp.tile([128, 2048], F32, tag="ps")
                            for rr in range(4):
                                g = (rr % 2) * HID
                                nc.tensor.matmul(
                                    ps[:, rr * 512:(rr + 1) * 512],
                                    sh_dup[g:g + HID, m * 128:(m + 1) * 128],
                                    h2T_dup[g:g + HID,
                                            p * 4096 + (rq * 4 + rr) * 512:
                                            p * 4096 + (rq * 4 + rr + 1) * 512],
                                    start=True, stop=True,
                                    tile_position=(g, 0),
                                    skip_group_check=True)
                            nc.scalar.activation(st[:], ps[:], AF.Sigmoid)
                            nc.sync.dma_start(
                                out4[m * 128:(m + 1) * 128,
                                     rq * 4:(rq + 1) * 4, p, :],
                                st.rearrange("m (r j) -> m r j", j=512))

    nc.compile()
    return nc


def _get_program():
    if "nc" not in _COMPILED:
        _COMPILED["nc"] = _build_program()
    return _COMPILED["nc"]


def _prep_inputs(x, edge_index, W1, b1, W2, b2):
    x = np.asarray(x, np.float32)
    ei = np.asarray(edge_index)
    src = ei[0].astype(np.int64)
    dst = ei[1].astype(np.int64)

    deg = (np.bincount(dst, minlength=N) + 1).astype(np.float32)
    degt = np.ascontiguousarray(deg.reshape(NCH, 128).T)           # [128, 64]

    xw = np.zeros((128, XW_COLS), dtype=ml_dtypes.bfloat16)
    xw[:, 0:HID] = np.asarray(W1, np.float32).astype(ml_dtypes.bfloat16)
    xw[0:HID, HID:2 * HID] = (
        np.asarray(W2, np.float32).astype(ml_dtypes.bfloat16))
    xw[:, 2 * HID:] = x.T.astype(ml_dtypes.bfloat16)

    b1c = np.asarray(b1, np.float32).reshape(HID)
    b2c = np.asarray(b2, np.float32).reshape(HID)

    in_maps = []
    for i in range(CORES):
        lo = i * DSH
        sel = (dst >= lo) & (dst < lo + DSH)
        flat = src[sel] * DSH + (dst[sel] - lo)
        cnt = np.bincount(flat, minlength=N * DSH).reshape(N, DSH)
        cnt[np.arange(lo, lo + DSH), np.arange(DSH)] += 1          # + I shard
        # SBUF layout: partition p holds src rows {c*128+p}, free = c*DSH + d
        A8 = np.ascontiguousarray(
            cnt.reshape(NCH, 128, DSH).transpose(1, 0, 2)
        ).astype(ml_dtypes.float8_e4m3).reshape(128, NCH * DSH)

        c32 = np.zeros((128, C_COLS), dtype=np.float32)
        c32[:, C_DEGT:C_DEGT + NCH] = degt
        c32[:, C_DEGSH:C_DEGSH + KSH] = deg[lo:lo + DSH].reshape(KSH, 128).T
        degb = np.broadcast_to(deg[lo:lo + DSH][None, :], (HID, DSH))
        c32[:, C_DEGB:C_DEGB + 512] = (
            degb.reshape(HID, 2, 512).transpose(1, 0, 2).reshape(128, 512))
        c32[0:HID, C_B1] = b1c
        c32[0:HID, C_B2] = b2c
        # rsqrt pipeline runs over every deg column; keep the b columns out
        # of it but the whole c32 tile must be finite for the Newton step.
        in_maps.append({"xw": xw, "A": A8, "c32": c32})
    return in_maps


def kernel(x, edge_index, W1, b1, W2, b2, _trace=False, _trace_kwargs=None):
    nc = _get_program()
    in_maps = _prep_inputs(x, edge_index, W1, b1, W2, b2)
    res = run_bass_kernel_spmd(nc, in_maps, core_ids=list(range(CORES)),
                               trace=_trace, **(_trace_kwargs or {}))
    out = np.concatenate([res.results[i]["out"] for i in range(CORES)], axis=0)
    if _trace:
        kernel._last_results = res
    return out.astype(np.float32)


# revision 22
# speedup vs baseline: 2.3352x; 1.0600x over previous
"""Trainium2 Bass kernel for a 2-layer GCN + sigmoid similarity matrix.

Model (see reference):
    h1 = relu(gcn_conv(x, W1, b1));  h2 = relu(gcn_conv(h1, W2, b2))
    out = sigmoid(h2 @ h2.T)                               # [8192, 8192]

gcn_conv(x, W, b) with self-loops and symmetric deg^{-1/2} norm factorizes:
    h  = x @ W
    out[d] = dinv[d] * sum_s Ahat[s, d] * (dinv[s] * h[s]) + b
where Ahat = edge-count matrix + I and dinv = rsqrt(indeg + 1).

Distribution over 8 NeuronCores (dst-sharded, per the sharding hint):
  - Every core computes hs1 = dinv * (x @ W1) for ALL nodes (cheap, replicated)
  - Ahat is densified per core as the [8192 src, 1024 dst] column shard, stored
    fp8_e4m3 (exact small integer counts) -> 8.4MB resident in SBUF.
  - Aggregation is a PE matmul: aggT[f, d] = sum_s hs[s, f] * Ahat[s, d],
    accumulated over 64 src chunks of 128 (lhsT = hs chunk bf16, rhs = A fp8),
    with even/odd chunks in separate PE column groups (concurrent matmuls).
  - Layer-1 output stays feature-major ([64, 1024] bf16 shard). Each core then
    computes hs2 = dinv * (h1 @ W2) for its own shard only and AllGathers the
    row-major [8192, 64] hs2 table, which feeds layer-2 aggregation directly.
  - h2 shards are AllGathered feature-major into [64, 8192]; each core computes
    its [1024, 8192] block of sigmoid(h2 @ h2.T) (bf16 PE matmul K=64 +
    ScalarE sigmoid from PSUM), written as bf16 and upcast to f32 on the host
    (sigmoid outputs here are ~0.52..0.60; bf16 costs ~1e-3 abs err).

Notes: all TensorEngine operands are bf16/fp8 (fp32 matmul runs as two PE
passes); a tiny first AllGather starts the collectives entry barrier (~45us)
early so it overlaps the input DMAs; constants ride one packed f32 tensor and
the weights ride in the x tensor so the startup issues few DMAs.
"""

import os
import sys

# bass/concourse toolchain location (not a problem-statement file)
for _p in ("/opt/trn_rl_repo", "/root/.axon_site/_ro/trn_rl_repo"):
    if os.path.isdir(_p) and _p not in sys.path:
        sys.path.insert(0, _p)
        break

# A cpu-forced JAX would hide the axon-tunneled NeuronCores this kernel needs.
if os.environ.get("JAX_PLATFORMS", "").strip().lower() in ("cpu",):
    os.environ.pop("JAX_PLATFORMS")

import numpy as np
import ml_dtypes

import concourse.bass as bass
import concourse.bacc as bacc
import concourse.mybir as mybir
from concourse import tile
from concourse.bass_utils import run_bass_kernel_spmd

N = 8192          # nodes
E = 262144        # edges
IN_DIM = 128
HID = 64
CORES = 8
DSH = N // CORES  # dst shard size (1024)
NCH = N // 128    # src chunks of 128 (64)
KSH = DSH // 128  # chunks per shard (8)

# packed f32 const tensor columns: degt | degsh | degb2 | b1 | b2
C_DEGT = 0
C_DEGSH = NCH                 # 64
C_DEGB = NCH + KSH            # 72
C_B1 = C_DEGB + 512           # 584
C_B2 = C_B1 + 1               # 585
C_COLS = C_B2 + 1             # 586
# packed bf16 tensor columns: W1 | W2(padded) | xT
XW_COLS = HID + HID + N

F32 = mybir.dt.float32
BF16 = mybir.dt.bfloat16
FP8 = mybir.dt.float8e4
AF = mybir.ActivationFunctionType
ALU = mybir.AluOpType

_COMPILED = {}


def _build_program():
    nc = bacc.Bacc("TRN2", target_bir_lowering=False, debug=False,
                   num_devices=CORES)

    # ---- I/O ----
    xw_d = nc.dram_tensor("xw", [128, XW_COLS], BF16, kind="ExternalInput")
    A_d = nc.dram_tensor("A", [128, NCH * DSH], FP8, kind="ExternalInput")
    c32_d = nc.dram_tensor("c32", [128, C_COLS], F32, kind="ExternalInput")
    out_d = nc.dram_tensor("out", [DSH, N], BF16, kind="ExternalOutput")

    with tile.TileContext(nc) as tc:
        with tc.tile_pool(name="const", bufs=1) as cpool, \
             tc.tile_pool(name="amat", bufs=1) as apool, \
             tc.tile_pool(name="dram", bufs=1, space="DRAM") as dpool:

            # Tiny first collective: starts the entry barrier + ncfw wakeup
            # immediately, overlapping the input DMAs. Must be cheap — it
            # serializes ahead of the first real AllGather on the CC stream.
            warm_sb = cpool.tile([64, 16], BF16)
            nc.gpsimd.memset(warm_sb[:], 0.0)
            warm_in = dpool.tile([64, 16], BF16)
            warm_out = dpool.tile([CORES * 64, 16], BF16)
            nc.gpsimd.dma_start(warm_in[:], warm_sb[:])
            nc.gpsimd.collective_compute(
                "AllGather", ALU.bypass,
                replica_groups=[[2 * g, 2 * g + 1] for g in range(CORES // 2)],
                ins=[warm_in.opt()], outs=[warm_out[0:128, :].opt()])

            # ---- packed constants (one DMA on the ACT HWDGE ring) ----
            c32 = cpool.tile([128, C_COLS], F32)
            nc.scalar.dma_start(c32[:], c32_d[:])
            b1_ap = c32[0:HID, C_B1:C_B1 + 1]
            b2_ap = c32[0:HID, C_B2:C_B2 + 1]

            # x + weights (bf16) on the ACT ring, first piece carries W1/W2.
            xw_sb = cpool.tile([128, XW_COLS], BF16)
            nc.scalar.dma_start(xw_sb[:, 0:1152], xw_d[:, 0:1152])
            for a in range(7):
                sl = slice(1152 + a * 1024, 1152 + (a + 1) * 1024)
                nc.scalar.dma_start(xw_sb[:, sl], xw_d[:, sl])
            W1_ap = xw_sb[:, 0:HID]
            W2_ap = xw_sb[0:HID, HID:2 * HID]

            def xT_chunk(c):
                return xw_sb[:, 2 * HID + c * 128: 2 * HID + (c + 1) * 128]

            # A (fp8, SBUF-resident) on the sync ring, in pieces.
            A_sb = apool.tile([128, NCH * DSH], FP8)
            APIECE = NCH // 8
            for a in range(8):
                sl = slice(a * APIECE * DSH, (a + 1) * APIECE * DSH)
                nc.sync.dma_start(A_sb[:, sl], A_d[:, sl])

            # ---- dinv = rsqrt(deg): fast reciprocal + sqrt + Newton step,
            # one fused pipeline over all packed deg columns ----
            DC = C_B1  # 584 deg columns
            deg_all = c32[:, 0:DC]
            r_ = cpool.tile([128, DC], F32)
            dinv = cpool.tile([128, DC], F32)
            t_ = cpool.tile([128, DC], F32)
            nc.vector.reciprocal_approx_fast(r_[:], deg_all)
            nc.scalar.activation(dinv[:], r_[:], AF.Sqrt)
            nc.vector.tensor_mul(t_[:], dinv[:], dinv[:])
            nc.vector.tensor_mul(t_[:], t_[:], deg_all)
            nc.vector.tensor_scalar(t_[:], t_[:], -0.5, 1.5, ALU.mult, ALU.add)
            nc.vector.tensor_mul(dinv[:], dinv[:], t_[:])
            dinvt = dinv[:, C_DEGT:C_DEGT + NCH]       # [128, 64]
            dinvsh = dinv[:, C_DEGSH:C_DEGSH + KSH]    # [128, 8]
            dinvb2 = dinv[:, C_DEGB:C_DEGB + 512]      # [128, 512] (2x64 halves)

            h1T_shard = cpool.tile([HID, DSH], BF16)
            h2T_shard = cpool.tile([HID, DSH], BF16)

            def linear_scaled(hs_sb, n_groups, make_lhsT, W_ap, dinv_ap):
                """hs_sb = dinv * (prev @ W), groups of 8 chunks per psum."""
                with tc.tile_pool(name="ph_psum", bufs=3, space="PSUM") as pp:
                    for g in range(n_groups):
                        ph = pp.tile([128, 8 * HID], F32, tag="ph")
                        for k in range(8):
                            nc.tensor.matmul(
                                ph[:, k * HID:(k + 1) * HID],
                                make_lhsT(g * 8 + k), W_ap,
                                start=True, stop=True)
                        dv = dinv_ap[:, g * 8:(g + 1) * 8]
                        nc.vector.tensor_tensor(
                            hs_sb.rearrange("p (c f) -> p c f", f=HID)
                                 [:, g * 8:(g + 1) * 8, :],
                            ph.rearrange("p (c f) -> p c f", f=HID),
                            dv.unsqueeze(2).broadcast_to((128, 8, HID)),
                            ALU.mult)

            def aggregate(hs_sb, b_ap, hT_out, order=None):
                """hT_out [64, DSH] bf16 = relu(dinv_d * (hs.T @ A) + b).
                The two dst halves accumulate concurrently in separate PE
                column groups (same stationary hs chunk loaded to both), so
                both finish as soon as the last chunk is consumed. `order`
                permutes the (sum-commutative) chunk visit order so chunks
                arriving from a split AllGather can be consumed first."""
                pairs = ([(c, c) for c in range(NCH)] if order is None
                         else order)  # (slot in hs_sb, chunk in A)
                with tc.tile_pool(name="ag_psum", bufs=1, space="PSUM") as gp, \
                     tc.tile_pool(name="ag_tmp", bufs=2) as tp:
                    pg = gp.tile([128, 512], F32, tag="pg")
                    for ci, (s, c) in enumerate(pairs):
                        for h in range(2):
                            nc.tensor.matmul(
                                pg[h * HID:(h + 1) * HID, :],
                                hs_sb[:, s * HID:(s + 1) * HID],
                                A_sb[:, c * DSH + h * 512:
                                     c * DSH + (h + 1) * 512],
                                start=(ci == 0), stop=(ci == NCH - 1),
                                tile_position=(0, h * HID),
                                skip_group_check=True)
                    for h in range(2):
                        tmp = tp.tile([HID, 512], F32, tag="tmp")
                        nc.vector.tensor_mul(tmp[:], pg[h * HID:(h + 1) * HID, :],
                                             dinvb2[h * HID:(h + 1) * HID, :])
                        nc.scalar.activation(hT_out[:, h * 512:(h + 1) * 512],
                                             tmp[:], AF.Relu, bias=b_ap)

            # ---- layer 1: hs1 for ALL nodes (replicated), aggregate shard --
            with tc.tile_pool(name="l1", bufs=1) as l1pool:
                hs1 = l1pool.tile([128, NCH * HID], BF16)
                linear_scaled(hs1, NCH // 8, xT_chunk, W1_ap, dinvt)
                aggregate(hs1, b1_ap, h1T_shard)

            # ---- hs2 for OWN shard, allgather row-major, layer 2 ----------
            with tc.tile_pool(name="l2", bufs=1) as l2pool:
                hs2_sh = l2pool.tile([128, KSH * HID], BF16)
                linear_scaled(hs2_sh, 1,
                              lambda k: h1T_shard[:, k * 128:(k + 1) * 128],
                              W2_ap, dinvsh)
                # AllGather hs2 in two halves (first/last 4 chunks of each
                # shard) so layer-2 aggregation starts on the first half
                # while the second is still in flight. hs2 slots are stored
                # in AG arrival order: slot = hh*32 + r*4 + k for node chunk
                # c = r*8 + hh*4 + k.
                hs2 = l2pool.tile([128, NCH * HID], BF16)
                for hh in range(2):
                    agin = dpool.tile([DSH // 2, HID], BF16, name=f"ag2in{hh}")
                    agout = dpool.tile([N // 2, HID], BF16,
                                       addr_space="Shared", name=f"ag2out{hh}")
                    nc.gpsimd.dma_start(
                        agin.rearrange("(k p) f -> p k f", p=128),
                        hs2_sh.rearrange("p (k f) -> p k f", f=HID)
                             [:, hh * 4:(hh + 1) * 4, :])
                    nc.gpsimd.collective_compute(
                        "AllGather", ALU.bypass,
                        replica_groups=[list(range(CORES))],
                        ins=[agin.opt()], outs=[agout.opt()])
                    nc.sync.dma_start(
                        hs2[:, hh * 2048:(hh + 1) * 2048]
                            .rearrange("p (q f) -> p q f", f=HID),
                        agout.rearrange("(q p) f -> p q f", p=128))
                order = [(hh * 32 + r * 4 + k, r * 8 + hh * 4 + k)
                         for hh in range(2) for r in range(CORES)
                         for k in range(4)]
                aggregate(hs2, b2_ap, h2T_shard, order=order)

            # ---- allgather h2 feature-major (two halves), sim + sigmoid ---
            # h2T is duplicated onto partitions 64:128 so pairs of j-tiles
            # run as concurrent K=64 matmuls in separate PE row groups.
            with tc.tile_pool(name="sim", bufs=1) as spool, \
                 tc.tile_pool(name="sim_psum", bufs=2, space="PSUM") as sp, \
                 tc.tile_pool(name="stage", bufs=4) as stpool:
                sh_dup = spool.tile([128, DSH], BF16)
                nc.scalar.dma_start(sh_dup[0:HID, :], h2T_shard[:])
                nc.scalar.dma_start(sh_dup[HID:128, :], h2T_shard[:])
                # free-dim layout of h2T_dup: pass p block at p*4096, then
                # rank r strip of 512 (= h2 cols r*1024 + p*512 + [0, 512))
                h2T_dup = spool.tile([128, N], BF16)
                for p in range(2):
                    agin = dpool.tile([HID, 512], BF16, name=f"ag3in{p}")
                    agout = dpool.tile([CORES * HID, 512], BF16,
                                       addr_space="Shared", name=f"ag3out{p}")
                    nc.gpsimd.dma_start(agin[:],
                                        h2T_shard[:, p * 512:(p + 1) * 512])
                    nc.gpsimd.collective_compute(
                        "AllGather", ALU.bypass,
                        replica_groups=[list(range(CORES))],
                        ins=[agin.opt()], outs=[agout.opt()])
                    src = agout.rearrange("(r f) j -> f r j", f=HID)
                    blk = h2T_dup[:, p * 4096:(p + 1) * 4096]
                    nc.scalar.dma_start(
                        blk[0:HID, :].rearrange("f (r j) -> f r j", j=512), src)
                    nc.scalar.dma_start(
                        blk[HID:128, :].rearrange("f (r j) -> f r j", j=512),
                        src)

                out4 = out_d.rearrange("m (r p j) -> m r p j", p=2, j=512)
                for p in range(2):
                    for m in range(DSH // 128):
                        for rq in range(2):
                            st = stpool.tile([128, 2048], BF16, tag="st")
                            ps = sp.tile([128, 2048], F32, tag="ps")
                            for rr in range(4):
                                g = (rr % 2) * HID
                                nc.tensor.matmul(
                                    ps[:, rr * 512:(rr + 1) * 512],
                                    sh_dup[g:g + HID, m * 128:(m + 1) * 128],
                                    h2T_dup[g:g + HID,
                                            p * 4096 + (rq * 4 + rr) * 512:
                                            p * 4096 + (rq * 4 + rr + 1) * 512],
                                    start=True, stop=True,
                                    tile_position=(g, 0),
                                    skip_group_check=True)
                            nc.scalar.activation(st[:], ps[:], AF.Sigmoid)
                            nc.sync.dma_start(
                                out4[m * 128:(m + 1) * 128,
                                     rq * 4:(rq + 1) * 4, p, :],
                                st.rearrange("m (r j) -> m r j", j=512))

    nc.compile()
    return nc


def _get_program():
    if "nc" not in _COMPILED:
        _COMPILED["nc"] = _build_program()
    return _COMPILED["nc"]


def _prep_inputs(x, edge_index, W1, b1, W2, b2):
    x = np.asarray(x, np.float32)
    ei = np.asarray(edge_index)
    src = ei[0].astype(np.int64)
    dst = ei[1].astype(np.int64)

    deg = (np.bincount(dst, minlength=N) + 1).astype(np.float32)
    degt = np.ascontiguousarray(deg.reshape(NCH, 128).T)           # [128, 64]

    xw = np.zeros((128, XW_COLS), dtype=ml_dtypes.bfloat16)
    xw[:, 0:HID] = np.asarray(W1, np.float32).astype(ml_dtypes.bfloat16)
    xw[0:HID, HID:2 * HID] = (
        np.asarray(W2, np.float32).astype(ml_dtypes.bfloat16))
    xw[:, 2 * HID:] = x.T.astype(ml_dtypes.bfloat16)

    b1c = np.asarray(b1, np.float32).reshape(HID)
    b2c = np.asarray(b2, np.float32).reshape(HID)

    in_maps = []
    for i in range(CORES):
        lo = i * DSH
        sel = (dst >= lo) & (dst < lo + DSH)
        flat = src[sel] * DSH + (dst[sel] - lo)
        cnt = np.bincount(flat, minlength=N * DSH).reshape(N, DSH)
        cnt[np.arange(lo, lo + DSH), np.arange(DSH)] += 1          # + I shard
        # SBUF layout: partition p holds src rows {c*128+p}, free = c*DSH + d
        A8 = np.ascontiguousarray(
            cnt.reshape(NCH, 128, DSH).transpose(1, 0, 2)
        ).astype(ml_dtypes.float8_e4m3).reshape(128, NCH * DSH)

        c32 = np.zeros((128, C_COLS), dtype=np.float32)
        c32[:, C_DEGT:C_DEGT + NCH] = degt
        c32[:, C_DEGSH:C_DEGSH + KSH] = deg[lo:lo + DSH].reshape(KSH, 128).T
        degb = np.broadcast_to(deg[lo:lo + DSH][None, :], (HID, DSH))
        c32[:, C_DEGB:C_DEGB + 512] = (
            degb.reshape(HID, 2, 512).transpose(1, 0, 2).reshape(128, 512))
        c32[0:HID, C_B1] = b1c
        c32[0:HID, C_B2] = b2c
        # rsqrt pipeline runs over every deg column; keep the b columns out
        # of it but the whole c32 tile must be finite for the Newton step.
        in_maps.append({"xw": xw, "A": A8, "c32": c32})
    return in_maps


def kernel(x, edge_index, W1, b1, W2, b2, _trace=False, _trace_kwargs=None):
    nc = _get_program()
    in_maps = _prep_inputs(x, edge_index, W1, b1, W2, b2)
    res = run_bass_kernel_spmd(nc, in_maps, core_ids=list(range(CORES)),
                               trace=_trace, **(_trace_kwargs or {}))
    out = np.concatenate([res.results[i]["out"] for i in range(CORES)], axis=0)
    if _trace:
        kernel._last_results = res
    return out.astype(np.float32)


# revision 23
# speedup vs baseline: 2.3845x; 1.0211x over previous
"""Trainium2 Bass kernel for a 2-layer GCN + sigmoid similarity matrix.

Model (see reference):
    h1 = relu(gcn_conv(x, W1, b1));  h2 = relu(gcn_conv(h1, W2, b2))
    out = sigmoid(h2 @ h2.T)                               # [8192, 8192]

gcn_conv(x, W, b) with self-loops and symmetric deg^{-1/2} norm factorizes:
    h  = x @ W
    out[d] = dinv[d] * sum_s Ahat[s, d] * (dinv[s] * h[s]) + b
where Ahat = edge-count matrix + I and dinv = rsqrt(indeg + 1).

Distribution over 8 NeuronCores (dst-sharded, per the sharding hint):
  - Every core computes hs1 = dinv * (x @ W1) for ALL nodes (cheap, replicated)
  - Ahat is densified per core as the [8192 src, 1024 dst] column shard, stored
    fp8_e4m3 (exact small integer counts) -> 8.4MB resident in SBUF.
  - Aggregation is a PE matmul: aggT[f, d] = sum_s hs[s, f] * Ahat[s, d],
    accumulated over 64 src chunks of 128 (lhsT = hs chunk bf16, rhs = A fp8),
    with even/odd chunks in separate PE column groups (concurrent matmuls).
  - Layer-1 output stays feature-major ([64, 1024] bf16 shard). Each core then
    computes hs2 = dinv * (h1 @ W2) for its own shard only and AllGathers the
    row-major [8192, 64] hs2 table, which feeds layer-2 aggregation directly.
  - h2 shards are AllGathered feature-major into [64, 8192]; each core computes
    its [1024, 8192] block of sigmoid(h2 @ h2.T) (bf16 PE matmul K=64 +
    ScalarE sigmoid from PSUM), written as bf16 and upcast to f32 on the host
    (sigmoid outputs here are ~0.52..0.60; bf16 costs ~1e-3 abs err).

Notes: all TensorEngine operands are bf16/fp8 (fp32 matmul runs as two PE
passes); a tiny first AllGather starts the collectives entry barrier (~45us)
early so it overlaps the input DMAs; constants ride one packed f32 tensor and
the weights ride in the x tensor so the startup issues few DMAs.
"""

import os
import sys

# bass/concourse toolchain location (not a problem-statement file)
for _p in ("/opt/trn_rl_repo", "/root/.axon_site/_ro/trn_rl_repo"):
    if os.path.isdir(_p) and _p not in sys.path:
        sys.path.insert(0, _p)
        break

# A cpu-forced JAX would hide the axon-tunneled NeuronCores this kernel needs.
if os.environ.get("JAX_PLATFORMS", "").strip().lower() in ("cpu",):
    os.environ.pop("JAX_PLATFORMS")

import numpy as np
import ml_dtypes

import concourse.bass as bass
import concourse.bacc as bacc
import concourse.mybir as mybir
from concourse import tile
from concourse.bass_utils import run_bass_kernel_spmd

N = 8192          # nodes
E = 262144        # edges
IN_DIM = 128
HID = 64
CORES = 8
DSH = N // CORES  # dst shard size (1024)
NCH = N // 128    # src chunks of 128 (64)
KSH = DSH // 128  # chunks per shard (8)

# packed f32 const tensor columns: degt | degsh | degb2 | b1 | b2
C_DEGT = 0
C_DEGSH = NCH                 # 64
C_DEGB = NCH + KSH            # 72
C_B1 = C_DEGB + 512           # 584
C_B2 = C_B1 + 1               # 585
C_COLS = C_B2 + 1             # 586
# packed bf16 tensor columns: W1 | W2(padded) | xT
XW_COLS = HID + HID + N

F32 = mybir.dt.float32
BF16 = mybir.dt.bfloat16
FP8 = mybir.dt.float8e4
AF = mybir.ActivationFunctionType
ALU = mybir.AluOpType

_COMPILED = {}


def _build_program():
    nc = bacc.Bacc("TRN2", target_bir_lowering=False, debug=False,
                   num_devices=CORES)

    # ---- I/O ----
    xw_d = nc.dram_tensor("xw", [128, XW_COLS], BF16, kind="ExternalInput")
    A_d = nc.dram_tensor("A", [128, NCH * DSH], FP8, kind="ExternalInput")
    c32_d = nc.dram_tensor("c32", [128, C_COLS], F32, kind="ExternalInput")
    out_d = nc.dram_tensor("out", [DSH, N], BF16, kind="ExternalOutput")

    with tile.TileContext(nc) as tc:
        with tc.tile_pool(name="const", bufs=1) as cpool, \
             tc.tile_pool(name="amat", bufs=1) as apool, \
             tc.tile_pool(name="dram", bufs=1, space="DRAM") as dpool:

            # Tiny first collective: starts the entry barrier + ncfw wakeup
            # immediately. Its input is an uninitialized DRAM tile (values
            # are never used) so the trigger has no data dependency and
            # fires right after the NEFF preamble, ahead of the input DMAs.
            warm_in = dpool.tile([64, 16], BF16)
            warm_out = dpool.tile([CORES * 64, 16], BF16)
            nc.gpsimd.collective_compute(
                "AllGather", ALU.bypass,
                replica_groups=[[2 * g, 2 * g + 1] for g in range(CORES // 2)],
                ins=[warm_in.opt()], outs=[warm_out[0:128, :].opt()])

            # ---- packed constants (one DMA on the ACT HWDGE ring) ----
            c32 = cpool.tile([128, C_COLS], F32)
            nc.scalar.dma_start(c32[:], c32_d[:])
            b1_ap = c32[0:HID, C_B1:C_B1 + 1]
            b2_ap = c32[0:HID, C_B2:C_B2 + 1]

            # x + weights (bf16) on the ACT ring, first piece carries W1/W2.
            xw_sb = cpool.tile([128, XW_COLS], BF16)
            nc.scalar.dma_start(xw_sb[:, 0:1152], xw_d[:, 0:1152])
            for a in range(7):
                sl = slice(1152 + a * 1024, 1152 + (a + 1) * 1024)
                nc.scalar.dma_start(xw_sb[:, sl], xw_d[:, sl])
            W1_ap = xw_sb[:, 0:HID]
            W2_ap = xw_sb[0:HID, HID:2 * HID]

            def xT_chunk(c):
                return xw_sb[:, 2 * HID + c * 128: 2 * HID + (c + 1) * 128]

            # A (fp8, SBUF-resident) on the sync ring, in pieces.
            A_sb = apool.tile([128, NCH * DSH], FP8)
            APIECE = NCH // 8
            for a in range(8):
                sl = slice(a * APIECE * DSH, (a + 1) * APIECE * DSH)
                nc.sync.dma_start(A_sb[:, sl], A_d[:, sl])

            # ---- dinv = rsqrt(deg): fast reciprocal + sqrt + Newton step,
            # one fused pipeline over all packed deg columns ----
            DC = C_B1  # 584 deg columns
            deg_all = c32[:, 0:DC]
            r_ = cpool.tile([128, DC], F32)
            dinv = cpool.tile([128, DC], F32)
            t_ = cpool.tile([128, DC], F32)
            nc.vector.reciprocal_approx_fast(r_[:], deg_all)
            nc.scalar.activation(dinv[:], r_[:], AF.Sqrt)
            nc.vector.tensor_mul(t_[:], dinv[:], dinv[:])
            nc.vector.tensor_mul(t_[:], t_[:], deg_all)
            nc.vector.tensor_scalar(t_[:], t_[:], -0.5, 1.5, ALU.mult, ALU.add)
            nc.vector.tensor_mul(dinv[:], dinv[:], t_[:])
            dinvt = dinv[:, C_DEGT:C_DEGT + NCH]       # [128, 64]
            dinvsh = dinv[:, C_DEGSH:C_DEGSH + KSH]    # [128, 8]
            dinvb2 = dinv[:, C_DEGB:C_DEGB + 512]      # [128, 512] (2x64 halves)

            h1T_shard = cpool.tile([HID, DSH], BF16)
            h2T_shard = cpool.tile([HID, DSH], BF16)

            def linear_scaled(hs_sb, n_groups, make_lhsT, W_ap, dinv_ap):
                """hs_sb = dinv * (prev @ W), groups of 8 chunks per psum."""
                with tc.tile_pool(name="ph_psum", bufs=3, space="PSUM") as pp:
                    for g in range(n_groups):
                        ph = pp.tile([128, 8 * HID], F32, tag="ph")
                        for k in range(8):
                            nc.tensor.matmul(
                                ph[:, k * HID:(k + 1) * HID],
                                make_lhsT(g * 8 + k), W_ap,
                                start=True, stop=True)
                        dv = dinv_ap[:, g * 8:(g + 1) * 8]
                        nc.vector.tensor_tensor(
                            hs_sb.rearrange("p (c f) -> p c f", f=HID)
                                 [:, g * 8:(g + 1) * 8, :],
                            ph.rearrange("p (c f) -> p c f", f=HID),
                            dv.unsqueeze(2).broadcast_to((128, 8, HID)),
                            ALU.mult)

            def aggregate(hs_sb, b_ap, hT_out, order=None):
                """hT_out [64, DSH] bf16 = relu(dinv_d * (hs.T @ A) + b).
                The two dst halves accumulate concurrently in separate PE
                column groups (same stationary hs chunk loaded to both), so
                both finish as soon as the last chunk is consumed. `order`
                permutes the (sum-commutative) chunk visit order so chunks
                arriving from a split AllGather can be consumed first."""
                pairs = ([(c, c) for c in range(NCH)] if order is None
                         else order)  # (slot in hs_sb, chunk in A)
                with tc.tile_pool(name="ag_psum", bufs=1, space="PSUM") as gp, \
                     tc.tile_pool(name="ag_tmp", bufs=2) as tp:
                    pg = gp.tile([128, 512], F32, tag="pg")
                    for ci, (s, c) in enumerate(pairs):
                        for h in range(2):
                            nc.tensor.matmul(
                                pg[h * HID:(h + 1) * HID, :],
                                hs_sb[:, s * HID:(s + 1) * HID],
                                A_sb[:, c * DSH + h * 512:
                                     c * DSH + (h + 1) * 512],
                                start=(ci == 0), stop=(ci == NCH - 1),
                                tile_position=(0, h * HID),
                                skip_group_check=True)
                    for h in range(2):
                        tmp = tp.tile([HID, 512], F32, tag="tmp")
                        nc.vector.tensor_mul(tmp[:], pg[h * HID:(h + 1) * HID, :],
                                             dinvb2[h * HID:(h + 1) * HID, :])
                        nc.scalar.activation(hT_out[:, h * 512:(h + 1) * 512],
                                             tmp[:], AF.Relu, bias=b_ap)

            # ---- layer 1: hs1 for ALL nodes (replicated), aggregate shard --
            with tc.tile_pool(name="l1", bufs=1) as l1pool:
                hs1 = l1pool.tile([128, NCH * HID], BF16)
                linear_scaled(hs1, NCH // 8, xT_chunk, W1_ap, dinvt)
                aggregate(hs1, b1_ap, h1T_shard)

            # ---- hs2 for OWN shard, allgather row-major, layer 2 ----------
            with tc.tile_pool(name="l2", bufs=1) as l2pool:
                hs2_sh = l2pool.tile([128, KSH * HID], BF16)
                linear_scaled(hs2_sh, 1,
                              lambda k: h1T_shard[:, k * 128:(k + 1) * 128],
                              W2_ap, dinvsh)
                # AllGather hs2 in two halves (first/last 4 chunks of each
                # shard) so layer-2 aggregation starts on the first half
                # while the second is still in flight. hs2 slots are stored
                # in AG arrival order: slot = hh*32 + r*4 + k for node chunk
                # c = r*8 + hh*4 + k.
                hs2 = l2pool.tile([128, NCH * HID], BF16)
                for hh in range(2):
                    agin = dpool.tile([DSH // 2, HID], BF16, name=f"ag2in{hh}")
                    agout = dpool.tile([N // 2, HID], BF16,
                                       addr_space="Shared", name=f"ag2out{hh}")
                    nc.gpsimd.dma_start(
                        agin.rearrange("(k p) f -> p k f", p=128),
                        hs2_sh.rearrange("p (k f) -> p k f", f=HID)
                             [:, hh * 4:(hh + 1) * 4, :])
                    nc.gpsimd.collective_compute(
                        "AllGather", ALU.bypass,
                        replica_groups=[list(range(CORES))],
                        ins=[agin.opt()], outs=[agout.opt()])
                    nc.sync.dma_start(
                        hs2[:, hh * 2048:(hh + 1) * 2048]
                            .rearrange("p (q f) -> p q f", f=HID),
                        agout.rearrange("(q p) f -> p q f", p=128))
                order = [(hh * 32 + r * 4 + k, r * 8 + hh * 4 + k)
                         for hh in range(2) for r in range(CORES)
                         for k in range(4)]
                aggregate(hs2, b2_ap, h2T_shard, order=order)

            # ---- allgather h2 feature-major (two halves), sim + sigmoid ---
            # h2T is duplicated onto partitions 64:128 so pairs of j-tiles
            # run as concurrent K=64 matmuls in separate PE row groups.
            with tc.tile_pool(name="sim", bufs=1) as spool, \
                 tc.tile_pool(name="sim_psum", bufs=2, space="PSUM") as sp, \
                 tc.tile_pool(name="stage", bufs=4) as stpool:
                sh_dup = spool.tile([128, DSH], BF16)
                nc.scalar.dma_start(sh_dup[0:HID, :], h2T_shard[:])
                nc.scalar.dma_start(sh_dup[HID:128, :], h2T_shard[:])
                # free-dim layout of h2T_dup: pass p block at p*4096, then
                # rank r strip of 512 (= h2 cols r*1024 + p*512 + [0, 512))
                h2T_dup = spool.tile([128, N], BF16)
                for p in range(2):
                    agin = dpool.tile([HID, 512], BF16, name=f"ag3in{p}")
                    agout = dpool.tile([CORES * HID, 512], BF16,
                                       addr_space="Shared", name=f"ag3out{p}")
                    nc.gpsimd.dma_start(agin[:],
                                        h2T_shard[:, p * 512:(p + 1) * 512])
                    nc.gpsimd.collective_compute(
                        "AllGather", ALU.bypass,
                        replica_groups=[list(range(CORES))],
                        ins=[agin.opt()], outs=[agout.opt()])
                    src = agout.rearrange("(r f) j -> f r j", f=HID)
                    blk = h2T_dup[:, p * 4096:(p + 1) * 4096]
                    nc.scalar.dma_start(
                        blk[0:HID, :].rearrange("f (r j) -> f r j", j=512), src)
                    nc.scalar.dma_start(
                        blk[HID:128, :].rearrange("f (r j) -> f r j", j=512),
                        src)

                out4 = out_d.rearrange("m (r p j) -> m r p j", p=2, j=512)
                for p in range(2):
                    for m in range(DSH // 128):
                        for rq in range(2):
                            st = stpool.tile([128, 2048], BF16, tag="st")
                            ps = sp.tile([128, 2048], F32, tag="ps")
                            for rr in range(4):
                                g = (rr % 2) * HID
                                nc.tensor.matmul(
                                    ps[:, rr * 512:(rr + 1) * 512],
                                    sh_dup[g:g + HID, m * 128:(m + 1) * 128],
                                    h2T_dup[g:g + HID,
                                            p * 4096 + (rq * 4 + rr) * 512:
                                            p * 4096 + (rq * 4 + rr + 1) * 512],
                                    start=True, stop=True,
                                    tile_position=(g, 0),
                                    skip_group_check=True)
                            nc.scalar.activation(st[:], ps[:], AF.Sigmoid)
                            nc.sync.dma_start(
                                out4[m * 128:(m + 1) * 128,
                                     rq * 4:(rq + 1) * 4, p, :],
                                st.rearrange("m (r j) -> m r j", j=512))

    nc.compile()
    return nc


def _get_program():
    if "nc" not in _COMPILED:
        _COMPILED["nc"] = _build_program()
    return _COMPILED["nc"]


def _prep_inputs(x, edge_index, W1, b1, W2, b2):
    x = np.asarray(x, np.float32)
    ei = np.asarray(edge_index)
    src = ei[0].astype(np.int64)
    dst = ei[1].astype(np.int64)

    deg = (np.bincount(dst, minlength=N) + 1).astype(np.float32)
    degt = np.ascontiguousarray(deg.reshape(NCH, 128).T)           # [128, 64]

    xw = np.zeros((128, XW_COLS), dtype=ml_dtypes.bfloat16)
    xw[:, 0:HID] = np.asarray(W1, np.float32).astype(ml_dtypes.bfloat16)
    xw[0:HID, HID:2 * HID] = (
        np.asarray(W2, np.float32).astype(ml_dtypes.bfloat16))
    xw[:, 2 * HID:] = x.T.astype(ml_dtypes.bfloat16)

    b1c = np.asarray(b1, np.float32).reshape(HID)
    b2c = np.asarray(b2, np.float32).reshape(HID)

    in_maps = []
    for i in range(CORES):
        lo = i * DSH
        sel = (dst >= lo) & (dst < lo + DSH)
        flat = src[sel] * DSH + (dst[sel] - lo)
        cnt = np.bincount(flat, minlength=N * DSH).reshape(N, DSH)
        cnt[np.arange(lo, lo + DSH), np.arange(DSH)] += 1          # + I shard
        # SBUF layout: partition p holds src rows {c*128+p}, free = c*DSH + d
        A8 = np.ascontiguousarray(
            cnt.reshape(NCH, 128, DSH).transpose(1, 0, 2)
        ).astype(ml_dtypes.float8_e4m3).reshape(128, NCH * DSH)

        c32 = np.zeros((128, C_COLS), dtype=np.float32)
        c32[:, C_DEGT:C_DEGT + NCH] = degt
        c32[:, C_DEGSH:C_DEGSH + KSH] = deg[lo:lo + DSH].reshape(KSH, 128).T
        degb = np.broadcast_to(deg[lo:lo + DSH][None, :], (HID, DSH))
        c32[:, C_DEGB:C_DEGB + 512] = (
            degb.reshape(HID, 2, 512).transpose(1, 0, 2).reshape(128, 512))
        c32[0:HID, C_B1] = b1c
        c32[0:HID, C_B2] = b2c
        # rsqrt pipeline runs over every deg column; keep the b columns out
        # of it but the whole c32 tile must be finite for the Newton step.
        in_maps.append({"xw": xw, "A": A8, "c32": c32})
    return in_maps


def kernel(x, edge_index, W1, b1, W2, b2, _trace=False, _trace_kwargs=None):
    nc = _get_program()
    in_maps = _prep_inputs(x, edge_index, W1, b1, W2, b2)
    res = run_bass_kernel_spmd(nc, in_maps, core_ids=list(range(CORES)),
                               trace=_trace, **(_trace_kwargs or {}))
    out = np.concatenate([res.results[i]["out"] for i in range(CORES)], axis=0)
    if _trace:
        kernel._last_results = res
    return out.astype(np.float32)
